# revision 4
# baseline (speedup 1.0000x reference)
"""Causal self-attention Trainium2 kernel (B=2, T=2048, D=1024, 16 heads).

Sharding: 8 cores = 2 batches x 4 head-groups (4 heads each).
Per core: column-parallel qkv, local attention, row-parallel proj producing a
partial output; host sums the 4 partials per batch and adds proj bias.
"""

import json
import math
import os

import numpy as np
import ml_dtypes

import bass_rust
import concourse.bass as bass
import concourse.bass2jax as bass2jax
import concourse.bass_utils as bass_utils
import concourse.mybir as mybir
import concourse.tile as tile
from concourse.tile import TileContext
from concourse.vector_clock import ScopedClock
from concourse.bass_utils import run_bass_kernel_spmd

BF16 = mybir.dt.bfloat16
F32 = mybir.dt.float32
NP_BF16 = ml_dtypes.bfloat16

D_MODEL = 1024
N_HEAD = 16
D_HEAD = 64
B = 2
T = 2048
G = 4                    # head-groups (tensor parallel)
HPC = N_HEAD // G        # heads per core = 4
QKC = 2 * HPC * D_HEAD   # q+k rows per core = 512
VC = HPC * D_HEAD        # v cols per core = 256
TCH = 512                # t-chunk (matmul moving free dim)
NTJ = T // TCH           # 4 t-chunks
NSI = T // 128           # 16 s-blocks
KC = D_MODEL // 128      # 8 contraction chunks over d_model
SCALE = 1.0 / math.sqrt(D_HEAD)


def _split_multi_waits(bir_json: bytes) -> bytes:
    """The walrus build in this container accepts at most one sync-wait
    command per instruction. Split instructions with N>1 waits into N-1
    preceding single-wait NoOps on the same engine (sequential waits AND
    together, so semantics are unchanged)."""
    bir = json.loads(bir_json)
    ctr = 0
    changed = False
    for func in bir.get("functions", []):
        for blk in func.get("blocks", []):
            out = []
            for inst in blk.get("instructions", []):
                si = inst.get("sync_info")
                waits = (si or {}).get("on_wait") or []
                if len(waits) > 1:
                    changed = True
                    for w in waits[:-1]:
                        ctr += 1
                        out.append(
                            {
                                "debug": inst.get("debug", 0),
                                "engine": inst.get("engine"),
                                "ins": [],
                                "name": f"WSPLIT-{ctr}",
                                "opcode": "NoOp",
                                "outs": [],
                                "sync_info": {"on_update": [], "on_wait": [w]},
                            }
                        )
                    si["on_wait"] = [waits[-1]]
                out.append(inst)
            blk["instructions"] = out
    if not changed:
        return bir_json
    return json.dumps(bir).encode()


_orig_compile_bir_kernel = bass_utils.compile_bir_kernel.__wrapped__ if hasattr(
    bass_utils.compile_bir_kernel, "__wrapped__"
) else bass_utils.compile_bir_kernel


def _patched_compile_bir_kernel(bir_json, tmpdir, neff_name="file.neff"):
    return _orig_compile_bir_kernel(_split_multi_waits(bir_json), tmpdir, neff_name)


def _patch_drain():
    """The walrus build in this container rejects >1 sync-wait command per
    instruction. Patch the compile path to split waits, and the TileContext
    terminal drain to emit single-wait SP nops."""
    if getattr(TileContext, "_drain_patched", False):
        return
    bass_utils.compile_bir_kernel = _patched_compile_bir_kernel
    bass2jax.compile_bir_kernel = _patched_compile_bir_kernel

    def _drain_and_barrier(self, tick_clock, wait_clock):
        nc = self.nc
        probe = nc.sync.nop()
        wait_clock.add_sem_waits(
            probe.ins, ScopedClock({None: tick_clock.global_clock})
        )
        si = probe.ins.sync_info
        waits = list(si.on_wait) if si is not None else []
        if si is not None:
            si.on_wait = waits[:1]
            probe.ins.sync_info = si
        for w in waits[1:]:
            n = nc.sync.nop()
            n.ins.sync_info = bass_rust.SyncInfo(on_wait=[w], on_update=[])
        nc.sync.drain()
        nc.all_engine_barrier()
        popped = nc._tile_sem_poison_stack.pop()
        assert popped is self._sem_poison
        nc.clear_and_free_semaphores(list(self.sems.allocated().values()))
        nc.all_engine_barrier()

    TileContext._drain_and_barrier = _drain_and_barrier
    TileContext._drain_patched = True


F8 = mybir.dt.float8e4
NP_F8 = ml_dtypes.float8_e4m3
S_W = 16.0                      # qkv weight prescale (fp8 subnormal avoidance)
SCALE_FP8 = SCALE / (S_W * S_W)  # folded into the exp activation
DR = mybir.MatmulPerfMode.DoubleRow


def _build_fp8():
    """Causal, no-bias path with fp8 DoubleRow matmuls.

    QKV: 3-term hi/lo split (hi*hi + lo*hi + hi*lo), DoubleRow pairs over
    adjacent d_model chunks -> 0.75x the bf16 matmul time.
    Scores: contraction stacked as [hi(64); lo(64)] on partitions.
      stationary KS pair0 = [Khi; Khi], pair1 = [Klo; 0]
      moving    QS = [Qhi; Qlo], broadcast along the pair dim
      -> one DR matmul per s-block = 2x the bf16 rate, 3-term accurate.
    PV / proj stay bf16.
    """
    _patch_drain()
    nc = bass.Bass()

    xh_d = nc.dram_tensor("xh", [D_MODEL, T], F8, kind="ExternalInput")
    xl_d = nc.dram_tensor("xl", [D_MODEL, T], F8, kind="ExternalInput")
    wh_d = nc.dram_tensor("wh", [D_MODEL, QKC + VC], F8, kind="ExternalInput")
    wl_d = nc.dram_tensor("wl", [D_MODEL, QKC + VC], F8, kind="ExternalInput")
    wp_d = nc.dram_tensor("wproj", [VC, D_MODEL], BF16, kind="ExternalInput")
    tri_d = nc.dram_tensor("tri", [128, 128], BF16, kind="ExternalInput")
    out_d = nc.dram_tensor("out", [T, D_MODEL], F32, kind="ExternalOutput")

    pre3 = os.environ.get("K_PRE3", "0") == "1"

    with TileContext(nc) as tc:
        with (
            tc.tile_pool(name="consts", bufs=1) as consts,
            tc.tile_pool(name="qkp", bufs=1) as qkp,
            tc.tile_pool(name="vp", bufs=1) as vp,
            tc.tile_pool(name="pp", bufs=int(os.environ.get("K_PP_BUFS", "2"))) as pp,
            tc.tile_pool(name="p3p", bufs=1) as p3p,
            tc.tile_pool(name="ap_", bufs=int(os.environ.get("K_AP_BUFS", "2"))) as ap_,
            tc.tile_pool(name="rp", bufs=int(os.environ.get("K_RP_BUFS", "2"))) as rp,
            tc.tile_pool(name="op_", bufs=int(os.environ.get("K_OSB_BUFS", "4"))) as op_,
            tc.tile_pool(name="sa_ps", bufs=int(os.environ.get("K_SA_BUFS", "2")), space="PSUM") as sa_ps_pool,
            tc.tile_pool(name="u_ps", bufs=int(os.environ.get("K_U_BUFS", "2")), space="PSUM") as u_ps_pool,
            tc.tile_pool(name="o_ps", bufs=int(os.environ.get("K_O_BUFS", "2")), space="PSUM") as o_ps_pool,
        ):
            # ---- constants ----
            Xh = consts.tile([128, KC, T], F8)
            Xl = consts.tile([128, KC, T], F8)
            Wh = consts.tile([128, KC, QKC + VC], F8)
            Wl = consts.tile([128, KC, QKC + VC], F8)
            WP = consts.tile([128, VC // 128, D_MODEL], BF16)
            tri = consts.tile([128, 128], BF16)
            # QS: per head [Qhi(p0:64); Qlo(p64:128)] over t
            QS = qkp.tile([128, HPC, T], F8)
            # KS: per head, pair0 = [Khi; Khi], pair1 = [Klo; 0]
            KS = qkp.tile([128, HPC, 2, T], F8)
            V = vp.tile([128, NSI, HPC, 2 * D_HEAD], BF16)

            xh_r = xh_d.rearrange("(c p) t -> p c t", p=128)
            xl_r = xl_d.rearrange("(c p) t -> p c t", p=128)
            wh_r = wh_d.rearrange("(c p) n -> p c n", p=128)
            wl_r = wl_d.rearrange("(c p) n -> p c n", p=128)
            NQ = int(os.environ.get("K_XQ", "4"))
            for c in range(KC):
                nc.gpsimd.dma_start(out=Wh[:, c], in_=wh_r[:, c])
            for q in range(NQ):
                lo, hi = q * (T // NQ), (q + 1) * (T // NQ)
                for c in range(KC):
                    nc.gpsimd.dma_start(out=Xh[:, c, lo:hi], in_=xh_r[:, c, lo:hi])
                if q == 0:
                    for c in range(KC):
                        nc.gpsimd.dma_start(out=Wl[:, c], in_=wl_r[:, c])
                for c in range(KC):
                    nc.gpsimd.dma_start(out=Xl[:, c, lo:hi], in_=xl_r[:, c, lo:hi])
            nc.scalar.dma_start(out=tri, in_=tri_d[:, :])
            nc.sync.dma_start(out=WP, in_=wp_d.rearrange("(c p) n -> p c n", p=128))
            # ones columns for the PV softmax-sum trick; zero region of KS pair1
            nc.gpsimd.memset(V[:, :, :, D_HEAD : 2 * D_HEAD], 1.0)
            nc.gpsimd.memset(KS[64:128, :, 1, :], 0.0)

            def emit_qkv_chunk(j):
                cols = slice(j * TCH, (j + 1) * TCH)
                # order q0, k0, q1, k1 so head 0/1 attention unblocks first
                for m in (0, 2, 1, 3):
                    ps = o_ps_pool.tile([128, TCH], F32, tag="ops")
                    nmm = 0
                    for w, x in ((Wh, Xh), (Wl, Xh), (Wh, Xl)):
                        for cp in range(KC // 2):
                            nmm += 1
                            nc.tensor.matmul(
                                ps,
                                w[:, 2 * cp : 2 * cp + 2, m * 128 : (m + 1) * 128],
                                x[:, 2 * cp : 2 * cp + 2, cols],
                                start=(nmm == 1),
                                stop=(nmm == 12),
                                perf_mode=DR,
                            )
                    if m < 2:
                        heads = (2 * m, 2 * m + 1)
                    else:
                        heads = (2 * (m - 2), 2 * (m - 2) + 1)
                    for u, hh in enumerate(heads):
                        pr = slice(64 * u, 64 * u + 64)
                        if m < 2:
                            nc.gpsimd.tensor_copy(
                                out=QS[0:64, hh, cols], in_=ps[pr]
                            )
                            nc.vector.tensor_tensor(
                                QS[64:128, hh, cols],
                                ps[pr],
                                QS[0:64, hh, cols],
                                mybir.AluOpType.subtract,
                            )
                        else:
                            nc.gpsimd.tensor_copy(
                                out=KS[0:64, hh, 0, cols], in_=ps[pr]
                            )
                            nc.vector.tensor_tensor(
                                KS[0:64, hh, 1, cols],
                                ps[pr],
                                KS[0:64, hh, 0, cols],
                                mybir.AluOpType.subtract,
                            )
                            nc.gpsimd.dma_start(
                                out=KS[64:128, hh, 0, cols],
                                in_=KS[0:64, hh, 0, cols],
                            )
                for i in range(4 * j, 4 * j + 4):
                    ps = o_ps_pool.tile([128, TCH], F32, tag="ops")
                    nmm = 0
                    for w, x in ((Wh, Xh), (Wl, Xh), (Wh, Xl)):
                        for cp in range(KC // 2):
                            nmm += 1
                            nc.tensor.matmul(
                                ps[:, :VC],
                                x[:, 2 * cp : 2 * cp + 2, i * 128 : (i + 1) * 128],
                                w[:, 2 * cp : 2 * cp + 2, QKC : QKC + VC],
                                start=(nmm == 1),
                                stop=(nmm == 12),
                                perf_mode=DR,
                            )
                    nc.gpsimd.tensor_copy(
                        out=V[:, i, :, 0:D_HEAD],
                        in_=ps[:, :VC].rearrange("p (h d) -> p h d", h=HPC),
                    )

            P3 = {}

            def emit_attention(tj, phase="full"):
                if phase != "scores":
                    A = ap_.tile([128, VC // 128, TCH], BF16)
                n_si = 4 * tj + 4
                for h in range(HPC):
                    if phase != "scores":
                        U = u_ps_pool.tile([2 * D_HEAD, TCH], F32, tag="u", name="U")
                    if phase == "scores":
                        P3[h] = p3p.tile([128, NSI, TCH], BF16, tag=f"p3h{h}", name="P3")
                        P = P3[h]
                    elif phase == "pv":
                        P = P3[h]
                    else:
                        P = pp.tile([128, NSI, TCH], BF16, tag="p", name="P")
                    n_sp = n_si // 2
                    for sp in range(n_sp):
                        spair = (2 * sp, 2 * sp + 1)
                        if phase != "pv":
                            s_ps = sa_ps_pool.tile([128, 2, TCH], F32, tag="s", name="s_ps")
                            for u_, si in enumerate(spair):
                                coff = 128 * (si - 4 * tj) if si >= 4 * tj else 0
                                ncols = TCH - coff
                                qs_mov = (
                                    QS[:, h, tj * TCH + coff : (tj + 1) * TCH]
                                    .unsqueeze(1)
                                    .broadcast_to([128, 2, ncols])
                                )
                                nc.tensor.matmul(
                                    s_ps[:, u_, coff:TCH],
                                    KS[:, h, :, si * 128 : (si + 1) * 128],
                                    qs_mov,
                                    start=True,
                                    stop=True,
                                    perf_mode=DR,
                                )
                            pcoff = 128 * (spair[0] - 4 * tj) if spair[0] >= 4 * tj else 0
                            nc.scalar.activation(
                                out=P[:, 2 * sp : 2 * sp + 2, pcoff:],
                                in_=s_ps[:, :, pcoff:],
                                func=mybir.ActivationFunctionType.Exp,
                                scale=SCALE_FP8,
                            )
                            for si in spair:
                                if si >= 4 * tj:
                                    coff = 128 * (si - 4 * tj)
                                    nc.gpsimd.tensor_tensor(
                                        P[:, si, coff : coff + 128],
                                        P[:, si, coff : coff + 128],
                                        tri,
                                        mybir.AluOpType.mult,
                                    )
                        if phase != "scores":
                            for si in spair:
                                coff = 128 * (si - 4 * tj) if si >= 4 * tj else 0
                                nc.tensor.matmul(
                                    U[:, coff:TCH],
                                    V[:, si, h, :],
                                    P[:, si, coff:TCH],
                                    start=(sp == 0 and si == spair[0]),
                                    stop=(sp == n_sp - 1 and si == spair[1]),
                                    skip_group_check=True,
                                )
                    if phase != "scores":
                        pb = 64 * (h % 2)
                        Rb = rp.tile([64, TCH], F32, tag="rbsb")
                        nc.vector.reciprocal(Rb, U[D_HEAD : 2 * D_HEAD, :])
                        a_slice = A[pb : pb + 64, h // 2, :]
                        nc.vector.tensor_tensor(
                            a_slice, U[0:D_HEAD, :], Rb, mybir.AluOpType.mult
                        )
                if phase == "scores":
                    return
                for tb in range(TCH // 128):
                    o_sb = op_.tile([128, D_MODEL], F32)
                    for n in range(D_MODEL // TCH):
                        o_ps = o_ps_pool.tile([128, TCH], F32, tag="ops")
                        for c in range(VC // 128):
                            nc.tensor.matmul(
                                o_ps,
                                A[:, c, tb * 128 : (tb + 1) * 128],
                                WP[:, c, n * TCH : (n + 1) * TCH],
                                start=(c == 0),
                                stop=(c == VC // 128 - 1),
                            )
                        eng = nc.vector if (tb + n) % 2 == 0 else nc.gpsimd
                        eng.tensor_copy(
                            out=o_sb[:, n * TCH : (n + 1) * TCH], in_=o_ps
                        )
                        nc.sync.dma_start(
                            out=out_d[
                                tj * TCH + tb * 128 : tj * TCH + (tb + 1) * 128,
                                n * TCH : (n + 1) * TCH,
                            ],
                            in_=o_sb[:, n * TCH : (n + 1) * TCH],
                        )

            cp = []
            for j in range(NTJ):
                emit_qkv_chunk(j)
                cp.append(tc.cur_priority)
            if pre3:
                for j in range(NTJ - 1):
                    off = tc.cur_priority - cp[j]
                    with tc.high_priority(offset=off):
                        emit_attention(j)
                off = tc.cur_priority - cp[NTJ - 1]
                with tc.high_priority(offset=off):
                    emit_attention(NTJ - 1, phase="scores")
                emit_attention(NTJ - 1, phase="pv")
            else:
                for j in range(NTJ):
                    off = tc.cur_priority - cp[j]
                    with tc.high_priority(offset=off):
                        emit_attention(j)
    return nc


def _build(mask_mode: str, has_qkv_bias: bool, head_pair=None, si_pair=None):
    """mask_mode: 'causal' | 'none' | 'generic'"""
    if head_pair is None:
        head_pair = os.environ.get("K_HEAD_PAIR", "0") == "1"
    if si_pair is None:
        si_pair = os.environ.get("K_SI_PAIR", "1") == "1"
    interleave = os.environ.get("K_INTERLEAVE", "0") == "1"
    _patch_drain()
    nc = bass.Bass()

    xT = nc.dram_tensor("xT", [D_MODEL, T], BF16, kind="ExternalInput")
    wqkv = nc.dram_tensor("wqkv", [D_MODEL, QKC + VC], BF16, kind="ExternalInput")
    wproj = nc.dram_tensor("wproj", [VC, D_MODEL], BF16, kind="ExternalInput")
    if mask_mode == "causal":
        tri_d = nc.dram_tensor("tri", [128, 128], BF16, kind="ExternalInput")
    if mask_mode == "generic":
        maskT_d = nc.dram_tensor("maskT", [T, T], BF16, kind="ExternalInput")
    if has_qkv_bias:
        bqk_d = nc.dram_tensor("bqk", [QKC], F32, kind="ExternalInput")
        bv_d = nc.dram_tensor("bv", [VC], F32, kind="ExternalInput")
    out_d = nc.dram_tensor("out", [T, D_MODEL], F32, kind="ExternalOutput")

    with TileContext(nc) as tc:
        with (
            tc.tile_pool(name="consts", bufs=1) as consts,
            tc.tile_pool(name="qkp", bufs=1) as qkp,
            tc.tile_pool(name="vp", bufs=1) as vp,
            tc.tile_pool(name="pp", bufs=int(os.environ.get("K_PP_BUFS", "2"))) as pp,
            tc.tile_pool(name="p3p", bufs=1) as p3p,
            tc.tile_pool(name="ap_", bufs=int(os.environ.get("K_AP_BUFS", "2"))) as ap_,
            tc.tile_pool(name="rp", bufs=int(os.environ.get("K_RP_BUFS", "2"))) as rp,
            tc.tile_pool(name="op_", bufs=int(os.environ.get("K_OSB_BUFS", "4"))) as op_,
            tc.tile_pool(name="dram_p", bufs=2, space="DRAM") as dram_p,
            tc.tile_pool(name="sa_ps", bufs=int(os.environ.get("K_SA_BUFS", str(4 // (2 if head_pair else 1) // (2 if si_pair else 1)))), space="PSUM") as sa_ps_pool,
            tc.tile_pool(name="sb_ps", bufs=(2 // (2 if si_pair else 1)), space="PSUM") as sb_ps_pool,
            tc.tile_pool(name="u_ps", bufs=int(os.environ.get("K_U_BUFS", "2")), space="PSUM") as u_ps_pool,
            tc.tile_pool(name="o_ps", bufs=int(os.environ.get("K_O_BUFS", "2")), space="PSUM") as o_ps_pool,
        ):
            # ---- load constants ----
            xT_r = xT.rearrange("(c p) t -> p c t", p=128)
            X = consts.tile([128, KC, T], BF16)
            wqkv_r = wqkv.rearrange("(c p) n -> p c n", p=128)
            W = consts.tile([128, KC, QKC + VC], BF16)
            dma_engs = [nc.sync, nc.gpsimd, nc.scalar]
            n_dma_eng = int(os.environ.get("K_DMA_ENGS", "3"))
            NQ = int(os.environ.get("K_XQ", "4"))
            for q in range(NQ):
                lo, hi = q * (T // NQ), (q + 1) * (T // NQ)
                for c in range(KC):
                    if q == 0:
                        dma_engs[c % n_dma_eng].dma_start(
                            out=W[:, c], in_=wqkv_r[:, c]
                        )
                    dma_engs[(q * KC + c + 1) % n_dma_eng].dma_start(
                        out=X[:, c, lo:hi], in_=xT_r[:, c, lo:hi]
                    )
            if mask_mode == "causal":
                tri = consts.tile([128, 128], BF16)
                nc.scalar.dma_start(out=tri, in_=tri_d[:, :])
            WP = consts.tile([128, VC // 128, D_MODEL], BF16)
            nc.sync.dma_start(out=WP, in_=wproj.rearrange("(c p) n -> p c n", p=128))
            if mask_mode == "generic":
                MT = consts.tile([128, NSI, T], BF16)
                nc.sync.dma_start(
                    out=MT, in_=maskT_d.rearrange("(si p) t -> p si t", p=128)
                )
            if has_qkv_bias:
                bqk = consts.tile([128, QKC // 128], F32)
                nc.sync.dma_start(
                    out=bqk, in_=bqk_d.rearrange("(m p) -> p m", p=128)
                )
                bv = consts.tile([128, VC // 128], F32)
                nc.sync.dma_start(out=bv, in_=bv_d.rearrange("(m p) -> p m", p=128))

            # V tile (natural layout). Each head gets 64 ones-columns
            # appended so the PV matmul (M=128, same pass cost as M=65)
            # emits the softmax sums replicated on partitions 64..127 --
            # the reciprocal+normalize then needs no partition broadcast.
            V = vp.tile([128, NSI, HPC, 2 * D_HEAD], BF16)
            nc.vector.memset(V[:, :, :, D_HEAD : 2 * D_HEAD], 1.0)
            QK = qkp.tile([128, QKC // 128, T], BF16)

            def emit_qkv_chunk(j):
                # Q^T / K^T chunk j: [qkrow, t] = sum_c W[c, qkrow] X^T[c, t]
                for m in range(QKC // 128):
                    qk_ps = o_ps_pool.tile([128, TCH], F32, tag="ops")
                    for c in range(KC):
                        nc.tensor.matmul(
                            qk_ps,
                            W[:, c, m * 128 : (m + 1) * 128],
                            X[:, c, j * TCH : (j + 1) * TCH],
                            start=(c == 0),
                            stop=(c == KC - 1),
                        )
                    if has_qkv_bias:
                        nc.scalar.activation(
                            out=QK[:, m, j * TCH : (j + 1) * TCH],
                            in_=qk_ps,
                            func=mybir.ActivationFunctionType.Identity,
                            bias=bqk[:, m : m + 1],
                        )
                    else:
                        nc.vector.tensor_copy(
                            out=QK[:, m, j * TCH : (j + 1) * TCH], in_=qk_ps
                        )
                # V rows for this chunk
                for i in range(4 * j, 4 * j + 4):
                    v_ps = o_ps_pool.tile([128, TCH], F32, tag="ops")
                    for c in range(KC):
                        nc.tensor.matmul(
                            v_ps[:, :VC],
                            X[:, c, i * 128 : (i + 1) * 128],
                            W[:, c, QKC : QKC + VC],
                            start=(c == 0),
                            stop=(c == KC - 1),
                        )
                    nc.vector.tensor_copy(
                        out=V[:, i, :, 0:D_HEAD],
                        in_=v_ps[:, :VC].rearrange("p (h d) -> p h d", h=HPC),
                    )

            pre3 = (
                os.environ.get("K_PRE3", "0") == "1" and mask_mode == "causal"
            )
            P3 = {}

            def emit_attention(tj, phase="full"):
                # phase: "full" | "scores" (S/exp/mask only, into P3 tiles)
                #        | "pv" (PV/norm/proj consuming P3 tiles)
                if phase != "scores":
                    A = ap_.tile([128, VC // 128, TCH], BF16)
                n_si = NSI if mask_mode != "causal" else 4 * tj + 4
                p_slices = 12 if pre3 else NSI
                HGRP = 2 if head_pair else 1
                SGRP = 2 if si_pair else 1
                def emit_head_group(hp):
                    heads = tuple(HGRP * hp + u for u in range(HGRP))
                    Us = {}
                    Ps = {}
                    for h in heads:
                        if phase != "scores":
                            Us[h] = u_ps_pool.tile(
                                [2 * D_HEAD, TCH], F32, tag="u", name="U"
                            )
                        if phase == "scores":
                            P3[h] = p3p.tile(
                                [128, NSI, TCH], BF16, tag=f"p3h{h}", name="P3"
                            )
                            Ps[h] = P3[h]
                        elif phase == "pv":
                            Ps[h] = P3[h]
                        else:
                            Ps[h] = pp.tile(
                                [128, p_slices, TCH], BF16, tag="p", name="P"
                            )
                    sp_order = list(range(n_si // SGRP))
                    if os.environ.get("K_SP_REV", "0") == "1":
                        sp_order = sp_order[::-1]
                    first_sp = sp_order[0]
                    last_sp = sp_order[-1]
                    for sp in sp_order:
                        spair = tuple(SGRP * sp + u for u in range(SGRP))
                        s_tiles = {}
                        for hi, h in enumerate(heads):
                            if phase == "pv":
                                break
                            pool = sa_ps_pool if hi == 0 else sb_ps_pool
                            s_ps = pool.tile([128, SGRP, TCH], F32, tag="s", name="s_ps")
                            s_tiles[h] = s_ps
                            pb = 64 * (h % 2)
                            qm = h // 2
                            km = 2 + h // 2
                            for u, si in enumerate(spair):
                                if mask_mode == "causal" and si >= 4 * tj:
                                    coff = 128 * (si - 4 * tj)
                                else:
                                    coff = 0
                                nc.tensor.matmul(
                                    s_ps[:, u, coff:TCH],
                                    QK[pb : pb + 64, km, si * 128 : (si + 1) * 128],
                                    QK[
                                        pb : pb + 64,
                                        qm,
                                        tj * TCH + coff : (tj + 1) * TCH,
                                    ],
                                    start=True,
                                    stop=True,
                                )
                        exp_split = (
                            os.environ.get("K_EXP_SPLIT", "0") == "1"
                            or tj >= int(os.environ.get("K_EXP_SPLIT_TJ", "99"))
                        )
                        for h in heads:
                            if phase == "pv":
                                break
                            # exp over the si-pair (prefixes of diagonal
                            # blocks hold garbage; never read back)
                            if exp_split:
                                for u in range(SGRP):
                                    nc.scalar.activation(
                                        out=Ps[h][:, SGRP * sp + u, :],
                                        in_=s_tiles[h][:, u, :],
                                        func=mybir.ActivationFunctionType.Exp,
                                        scale=SCALE,
                                    )
                            else:
                                if mask_mode == "causal" and spair[0] >= 4 * tj:
                                    pcoff = 128 * (spair[0] - 4 * tj)
                                else:
                                    pcoff = 0
                                nc.scalar.activation(
                                    out=Ps[h][:, SGRP * sp : SGRP * sp + SGRP, pcoff:],
                                    in_=s_tiles[h][:, :, pcoff:],
                                    func=mybir.ActivationFunctionType.Exp,
                                    scale=SCALE,
                                )
                            for si in spair:
                                if mask_mode == "causal" and si >= 4 * tj:
                                    coff = 128 * (si - 4 * tj)
                                    nc.vector.tensor_tensor(
                                        Ps[h][:, si, coff : coff + 128],
                                        Ps[h][:, si, coff : coff + 128],
                                        tri,
                                        mybir.AluOpType.mult,
                                    )
                            if mask_mode == "generic":
                                for si in spair:
                                    nc.vector.tensor_tensor(
                                        Ps[h][:, si, :],
                                        Ps[h][:, si, :],
                                        MT[:, si, tj * TCH : (tj + 1) * TCH],
                                        mybir.AluOpType.mult,
                                    )
                        for h in heads:
                            if phase == "scores":
                                break
                            for si in spair:
                                if mask_mode == "causal" and si >= 4 * tj:
                                    coff = 128 * (si - 4 * tj)
                                else:
                                    coff = 0
                                nc.tensor.matmul(
                                    Us[h][:, coff:TCH],
                                    V[:, si, h, :],
                                    Ps[h][:, si, coff:TCH],
                                    start=(sp == first_sp and si == spair[0]),
                                    stop=(sp == last_sp and si == spair[-1]),
                                    skip_group_check=True,
                                )
                    for h in heads:
                        if phase == "scores":
                            break
                        # normalize: sums sit replicated on partitions
                        # 64..127 of U; reciprocal them straight to SBUF
                        pb = 64 * (h % 2)
                        Rb_sb = rp.tile([64, TCH], F32, tag="rbsb")
                        nc.vector.reciprocal(Rb_sb, Us[h][D_HEAD : 2 * D_HEAD, :])
                        a_slice = A[pb : pb + 64, h // 2, :]
                        nc.vector.tensor_tensor(
                            a_slice, Us[h][0:D_HEAD, :], Rb_sb, mybir.AluOpType.mult
                        )
                        if has_qkv_bias:
                            nc.scalar.activation(
                                out=a_slice,
                                in_=a_slice,
                                func=mybir.ActivationFunctionType.Identity,
                                bias=bv[pb : pb + 64, h // 2 : h // 2 + 1],
                            )

                head_ilv = os.environ.get("K_HEAD_ILV", "0") == "1"
                for hp in range(HPC // HGRP):
                    if head_ilv and hp % 2 == 1:
                        off = tc.cur_priority - pair_base
                        with tc.high_priority(offset=off):
                            emit_head_group(hp)
                    else:
                        pair_base = tc.cur_priority
                        emit_head_group(hp)
                if phase == "scores":
                    return
                # proj for this t-chunk: out[t, n] = sum_c A^T[c, t] * WP[c, n]
                for tb in range(TCH // 128):
                    o_sb = op_.tile([128, D_MODEL], F32)
                    for n in range(D_MODEL // TCH):
                        o_ps = o_ps_pool.tile([128, TCH], F32, tag="ops")
                        for c in range(VC // 128):
                            nc.tensor.matmul(
                                o_ps,
                                A[:, c, tb * 128 : (tb + 1) * 128],
                                WP[:, c, n * TCH : (n + 1) * TCH],
                                start=(c == 0),
                                stop=(c == VC // 128 - 1),
                            )
                        nc.vector.tensor_copy(
                            out=o_sb[:, n * TCH : (n + 1) * TCH], in_=o_ps
                        )
                        nc.sync.dma_start(
                            out=out_d[
                                tj * TCH + tb * 128 : tj * TCH + (tb + 1) * 128,
                                n * TCH : (n + 1) * TCH,
                            ],
                            in_=o_sb[:, n * TCH : (n + 1) * TCH],
                        )

            prio_mode = os.environ.get("K_PRIO", "1") == "1"
            if interleave:
                for j in range(NTJ):
                    emit_qkv_chunk(j)
                    emit_attention(j)
            elif prio_mode:
                # emit qkv first (program order = dataflow order), but give
                # attention tj a priority window starting right after qkv
                # chunk tj, so the scheduler fills attention stalls with
                # later qkv chunks
                cp = []
                for j in range(NTJ):
                    emit_qkv_chunk(j)
                    cp.append(tc.cur_priority)
                if pre3:
                    # tj3's S/exp/mask precompute as mid-kernel filler
                    # (window right after qkv chunk 3); its PV/norm/proj
                    # run last as a dense pure-PE tail
                    for j in range(NTJ - 1):
                        off = tc.cur_priority - cp[j]
                        with tc.high_priority(offset=off):
                            emit_attention(j)
                    off = tc.cur_priority - cp[NTJ - 1]
                    with tc.high_priority(offset=off):
                        emit_attention(NTJ - 1, phase="scores")
                    emit_attention(NTJ - 1, phase="pv")
                else:
                    for j in range(NTJ):
                        off = tc.cur_priority - cp[j]
                        with tc.high_priority(offset=off):
                            emit_attention(j)
            else:
                for j in range(NTJ):
                    emit_qkv_chunk(j)
                for j in range(NTJ):
                    emit_attention(j)
    return nc


_NC_CACHE: dict = {}


def _use_fp8(mask_mode: str, has_qkv_bias: bool) -> bool:
    if os.environ.get("K_FP8", "1") != "1":
        return False
    return mask_mode == "causal" and not has_qkv_bias


def _get_nc(mask_mode: str, has_qkv_bias: bool):
    if _use_fp8(mask_mode, has_qkv_bias):
        key = "fp8"
        if key not in _NC_CACHE:
            _NC_CACHE[key] = _build_fp8()
        return _NC_CACHE[key]
    key = (mask_mode, has_qkv_bias)
    if key not in _NC_CACHE:
        _NC_CACHE[key] = _build(mask_mode, has_qkv_bias)
    return _NC_CACHE[key]


def classify_inputs(mask, qkv_b):
    m2 = np.asarray(mask).reshape(T, T)
    if np.array_equal(m2 != 0, np.tril(np.ones((T, T), dtype=bool))):
        mask_mode = "causal"
    elif np.all(m2 != 0):
        mask_mode = "none"
    else:
        mask_mode = "generic"
    has_qkv_bias = bool(np.any(np.asarray(qkv_b) != 0.0))
    return mask_mode, has_qkv_bias


def prepare_in_maps(x, mask, qkv_w, qkv_b, proj_w, proj_b):
    x = np.asarray(x, dtype=np.float32)
    qkv_w = np.asarray(qkv_w, dtype=np.float32)
    qkv_b = np.asarray(qkv_b, dtype=np.float32)
    proj_w = np.asarray(proj_w, dtype=np.float32)
    mask_mode, has_qkv_bias = classify_inputs(mask, qkv_b)
    m2 = np.asarray(mask).reshape(T, T)

    tri_np = np.triu(np.ones((128, 128))).astype(NP_BF16)
    in_maps = []
    for b in range(B):
        xT_b = np.ascontiguousarray(x[b].T).astype(NP_BF16)
        for g in range(G):
            qs = qkv_w[:, g * VC : (g + 1) * VC]
            ks = qkv_w[:, D_MODEL + g * VC : D_MODEL + (g + 1) * VC]
            vs = qkv_w[:, 2 * D_MODEL + g * VC : 2 * D_MODEL + (g + 1) * VC]
            im = {
                "xT": xT_b,
                "wqkv": np.ascontiguousarray(
                    np.concatenate([qs, ks, vs], axis=1)
                ).astype(NP_BF16),
                "wproj": np.ascontiguousarray(
                    proj_w[g * VC : (g + 1) * VC, :]
                ).astype(NP_BF16),
            }
            if mask_mode == "causal":
                im["tri"] = tri_np
            if mask_mode == "generic":
                im["maskT"] = np.ascontiguousarray(
                    (m2 != 0).T.astype(NP_BF16)
                )
            if has_qkv_bias:
                im["bqk"] = np.ascontiguousarray(
                    np.concatenate(
                        [qkv_b[g * VC : (g + 1) * VC],
                         qkv_b[D_MODEL + g * VC : D_MODEL + (g + 1) * VC]]
                    )
                ).astype(np.float32)
                im["bv"] = np.ascontiguousarray(
                    qkv_b[2 * D_MODEL + g * VC : 2 * D_MODEL + (g + 1) * VC]
                ).astype(np.float32)
            in_maps.append(im)
    return in_maps


def prepare_in_maps_fp8(x, qkv_w, proj_w):
    x = np.asarray(x, dtype=np.float32)
    qkv_w = np.asarray(qkv_w, dtype=np.float32) * S_W
    proj_w = np.asarray(proj_w, dtype=np.float32) / S_W
    tri_np = np.triu(np.ones((128, 128))).astype(NP_BF16)
    in_maps = []
    for b in range(B):
        xT = np.ascontiguousarray(x[b].T)
        xh = xT.astype(NP_F8)
        xl = (xT - xh.astype(np.float32)).astype(NP_F8)
        for g in range(G):
            qs = qkv_w[:, g * VC : (g + 1) * VC]
            ks = qkv_w[:, D_MODEL + g * VC : D_MODEL + (g + 1) * VC]
            vs = qkv_w[:, 2 * D_MODEL + g * VC : 2 * D_MODEL + (g + 1) * VC]
            w = np.ascontiguousarray(np.concatenate([qs, ks, vs], axis=1))
            wh = w.astype(NP_F8)
            wl = (w - wh.astype(np.float32)).astype(NP_F8)
            in_maps.append(
                {
                    "xh": xh,
                    "xl": xl,
                    "wh": wh,
                    "wl": wl,
                    "wproj": np.ascontiguousarray(
                        proj_w[g * VC : (g + 1) * VC, :]
                    ).astype(NP_BF16),
                    "tri": tri_np,
                }
            )
    return in_maps


def kernel(x, mask, qkv_w, qkv_b, proj_w, proj_b):
    proj_b = np.asarray(proj_b, dtype=np.float32)
    mask_mode, has_qkv_bias = classify_inputs(mask, qkv_b)
    nc = _get_nc(mask_mode, has_qkv_bias)
    if _use_fp8(mask_mode, has_qkv_bias):
        in_maps = prepare_in_maps_fp8(x, qkv_w, proj_w)
    else:
        in_maps = prepare_in_maps(x, mask, qkv_w, qkv_b, proj_w, proj_b)

    trace = bool(os.environ.get("KERNEL_TRACE"))
    res = run_bass_kernel_spmd(
        nc, in_maps, core_ids=list(range(B * G)), trace=trace
    )
    globals()["LAST_RESULT"] = res
    outs = [np.asarray(r["out"], dtype=np.float32) for r in res.results]

    final = np.empty((B, T, D_MODEL), dtype=np.float32)
    for b in range(B):
        acc = outs[b * G].copy()
        for g in range(1, G):
            acc += outs[b * G + g]
        final[b] = acc + proj_b[None, :]
    return final



# revision 12
# speedup vs baseline: 1.6582x; 1.6582x over previous
"""Causal self-attention Trainium2 kernel (B=2, T=2048, D=1024, 16 heads).

Sharding: 8 cores = 2 batches x 4 head-groups (4 heads each).
Per core: column-parallel qkv, local attention, row-parallel proj producing a
partial output; host sums the 4 partials per batch and adds proj bias.
"""

import json
import math
import os

import numpy as np
import ml_dtypes

import bass_rust
import concourse.bass as bass
import concourse.bass2jax as bass2jax
import concourse.bass_utils as bass_utils
import concourse.mybir as mybir
import concourse.tile as tile
from concourse.tile import TileContext
from concourse.vector_clock import ScopedClock
from concourse.bass_utils import run_bass_kernel_spmd

BF16 = mybir.dt.bfloat16
F32 = mybir.dt.float32
NP_BF16 = ml_dtypes.bfloat16

D_MODEL = 1024
N_HEAD = 16
D_HEAD = 64
B = 2
T = 2048
G = 4                    # head-groups (tensor parallel)
HPC = N_HEAD // G        # heads per core = 4
QKC = 2 * HPC * D_HEAD   # q+k rows per core = 512
VC = HPC * D_HEAD        # v cols per core = 256
TCH = 512                # t-chunk (matmul moving free dim)
NTJ = T // TCH           # 4 t-chunks
NSI = T // 128           # 16 s-blocks
KC = D_MODEL // 128      # 8 contraction chunks over d_model
SCALE = 1.0 / math.sqrt(D_HEAD)


def _split_multi_waits(bir_json: bytes) -> bytes:
    """The walrus build in this container accepts at most one sync-wait
    command per instruction. Split instructions with N>1 waits into N-1
    preceding single-wait NoOps on the same engine (sequential waits AND
    together, so semantics are unchanged)."""
    bir = json.loads(bir_json)
    ctr = 0
    changed = False
    for func in bir.get("functions", []):
        for blk in func.get("blocks", []):
            out = []
            for inst in blk.get("instructions", []):
                si = inst.get("sync_info")
                waits = (si or {}).get("on_wait") or []
                if len(waits) > 1:
                    changed = True
                    for w in waits[:-1]:
                        ctr += 1
                        out.append(
                            {
                                "debug": inst.get("debug", 0),
                                "engine": inst.get("engine"),
                                "ins": [],
                                "name": f"WSPLIT-{ctr}",
                                "opcode": "NoOp",
                                "outs": [],
                                "sync_info": {"on_update": [], "on_wait": [w]},
                            }
                        )
                    si["on_wait"] = [waits[-1]]
                out.append(inst)
            blk["instructions"] = out
    if not changed:
        return bir_json
    return json.dumps(bir).encode()


_orig_compile_bir_kernel = bass_utils.compile_bir_kernel.__wrapped__ if hasattr(
    bass_utils.compile_bir_kernel, "__wrapped__"
) else bass_utils.compile_bir_kernel


def _patched_compile_bir_kernel(bir_json, tmpdir, neff_name="file.neff"):
    return _orig_compile_bir_kernel(_split_multi_waits(bir_json), tmpdir, neff_name)


def _patch_drain():
    """The walrus build in this container rejects >1 sync-wait command per
    instruction. Patch the compile path to split waits, and the TileContext
    terminal drain to emit single-wait SP nops."""
    if getattr(TileContext, "_drain_patched", False):
        return
    bass_utils.compile_bir_kernel = _patched_compile_bir_kernel
    bass2jax.compile_bir_kernel = _patched_compile_bir_kernel

    def _drain_and_barrier(self, tick_clock, wait_clock):
        nc = self.nc
        probe = nc.sync.nop()
        wait_clock.add_sem_waits(
            probe.ins, ScopedClock({None: tick_clock.global_clock})
        )
        si = probe.ins.sync_info
        waits = list(si.on_wait) if si is not None else []
        if si is not None:
            si.on_wait = waits[:1]
            probe.ins.sync_info = si
        for w in waits[1:]:
            n = nc.sync.nop()
            n.ins.sync_info = bass_rust.SyncInfo(on_wait=[w], on_update=[])
        nc.sync.drain()
        nc.all_engine_barrier()
        popped = nc._tile_sem_poison_stack.pop()
        assert popped is self._sem_poison
        nc.clear_and_free_semaphores(list(self.sems.allocated().values()))
        nc.all_engine_barrier()

    TileContext._drain_and_barrier = _drain_and_barrier
    TileContext._drain_patched = True


F8 = mybir.dt.float8e4
NP_F8 = ml_dtypes.float8_e4m3
S_W = 16.0                      # qkv weight prescale (fp8 subnormal avoidance)
SCALE_FP8 = SCALE / (S_W * S_W)  # folded into the exp activation
DR = mybir.MatmulPerfMode.DoubleRow


def _build_fp8():
    """Causal, no-bias path with fp8 DoubleRow matmuls.

    QKV: 3-term hi/lo split (hi*hi + lo*hi + hi*lo), DoubleRow pairs over
    adjacent d_model chunks -> 0.75x the bf16 matmul time.
    Scores: contraction stacked as [hi(64); lo(64)] on partitions.
      stationary KS pair0 = [Khi; Khi], pair1 = [Klo; 0]
      moving    QS = [Qhi; Qlo], broadcast along the pair dim
      -> one DR matmul per s-block = 2x the bf16 rate, 3-term accurate.
    PV / proj stay bf16.
    """
    _patch_drain()
    nc = bass.Bass()

    xh_d = nc.dram_tensor("xh", [D_MODEL, T], F8, kind="ExternalInput")
    xl_d = nc.dram_tensor("xl", [D_MODEL, T], F8, kind="ExternalInput")
    wh_d = nc.dram_tensor("wh", [D_MODEL, QKC + VC], F8, kind="ExternalInput")
    wl_d = nc.dram_tensor("wl", [D_MODEL, QKC + VC], F8, kind="ExternalInput")
    wp_d = nc.dram_tensor("wproj", [VC, D_MODEL], BF16, kind="ExternalInput")
    tri_d = nc.dram_tensor("tri", [128, 128], BF16, kind="ExternalInput")
    ones_d = nc.dram_tensor("ones", [128, NSI * HPC * D_HEAD], BF16, kind="ExternalInput")
    zeros_d = nc.dram_tensor("zeros", [64, HPC * T], F8, kind="ExternalInput")
    out_d = nc.dram_tensor("out", [T, D_MODEL], F32, kind="ExternalOutput")

    pre3 = os.environ.get("K_PRE3", "1") == "1"

    with TileContext(nc) as tc:
        with (
            tc.tile_pool(name="consts", bufs=1) as consts,
            tc.tile_pool(name="qkp", bufs=1) as qkp,
            tc.tile_pool(name="vp", bufs=1) as vp,
            tc.tile_pool(name="pp", bufs=int(os.environ.get("K_PP_BUFS", "2"))) as pp,
            tc.tile_pool(name="p3p", bufs=1) as p3p,
            tc.tile_pool(name="ap_", bufs=int(os.environ.get("K_AP_BUFS", "2"))) as ap_,
            tc.tile_pool(name="rp", bufs=int(os.environ.get("K_RP_BUFS", "2"))) as rp,
            tc.tile_pool(name="op_", bufs=int(os.environ.get("K_OSB_BUFS", "3"))) as op_,
            tc.tile_pool(name="sa_ps", bufs=int(os.environ.get("K_SA_BUFS", "2")), space="PSUM") as sa_ps_pool,
            tc.tile_pool(name="u_ps", bufs=int(os.environ.get("K_U_BUFS", "2")), space="PSUM") as u_ps_pool,
            tc.tile_pool(name="o_ps", bufs=int(os.environ.get("K_O_BUFS", "2")), space="PSUM") as o_ps_pool,
        ):
            # ---- constants ----
            Xh = consts.tile([128, KC, T], F8)
            Xl = consts.tile([128, KC, T], F8)
            Wh = consts.tile([128, KC, QKC + VC], F8)
            Wl = consts.tile([128, KC, QKC + VC], F8)
            WP = consts.tile([128, VC // 128, D_MODEL], BF16)
            tri = consts.tile([128, 128], BF16)
            # QS: per head [Qhi(p0:64); Qlo(p64:128)] over t
            QS = qkp.tile([128, HPC, T], F8)
            # KS: per head, pair0 = [Khi; Khi], pair1 = [Klo; 0]
            KS = qkp.tile([128, HPC, 2, T], F8)
            V = vp.tile([128, NSI, HPC, 2 * D_HEAD], BF16)

            xh_r = xh_d.rearrange("(c p) t -> p c t", p=128)
            xl_r = xl_d.rearrange("(c p) t -> p c t", p=128)
            wh_r = wh_d.rearrange("(c p) n -> p c n", p=128)
            wl_r = wl_d.rearrange("(c p) n -> p c n", p=128)
            NQ = int(os.environ.get("K_XQ", "4"))
            nc.sync.dma_start(out=Wh, in_=wh_r)
            for q in range(NQ):
                lo, hi = q * (T // NQ), (q + 1) * (T // NQ)
                nc.sync.dma_start(out=Xh[:, :, lo:hi], in_=xh_r[:, :, lo:hi])
                if q == 0:
                    nc.sync.dma_start(out=Wl, in_=wl_r)
                nc.sync.dma_start(out=Xl[:, :, lo:hi], in_=xl_r[:, :, lo:hi])
            nc.scalar.dma_start(out=tri, in_=tri_d[:, :])
            nc.sync.dma_start(out=WP, in_=wp_d.rearrange("(c p) n -> p c n", p=128))
            # ones columns for the PV softmax-sum trick; zero region of KS pair1
            # (DMA'd, not memset: keeps Pool free during the startup window)
            nc.scalar.dma_start(
                out=V[:, :, :, D_HEAD : 2 * D_HEAD],
                in_=ones_d.rearrange("p (s h d) -> p s h d", s=NSI, h=HPC),
            )
            nc.scalar.dma_start(
                out=KS[64:128, :, 1, :],
                in_=zeros_d.rearrange("p (h t) -> p h t", h=HPC),
            )

            def emit_qkv_chunk(j):
                cols = slice(j * TCH, (j + 1) * TCH)
                # order q0, k0, q1, k1 so head 0/1 attention unblocks first
                for m in (0, 2, 1, 3):
                    ps = o_ps_pool.tile([128, TCH], F32, tag="ops")
                    nmm = 0
                    for w, x in ((Wh, Xh), (Wl, Xh), (Wh, Xl)):
                        for cp in range(KC // 2):
                            nmm += 1
                            nc.tensor.matmul(
                                ps,
                                w[:, 2 * cp : 2 * cp + 2, m * 128 : (m + 1) * 128],
                                x[:, 2 * cp : 2 * cp + 2, cols],
                                start=(nmm == 1),
                                stop=(nmm == 12),
                                perf_mode=DR,
                            )
                    if m < 2:
                        heads = (2 * m, 2 * m + 1)
                    else:
                        heads = (2 * (m - 2), 2 * (m - 2) + 1)
                    for u, hh in enumerate(heads):
                        pr = slice(64 * u, 64 * u + 64)
                        if m < 2:
                            nc.gpsimd.tensor_copy(
                                out=QS[0:64, hh, cols], in_=ps[pr]
                            )
                            nc.vector.tensor_tensor(
                                QS[64:128, hh, cols],
                                ps[pr],
                                QS[0:64, hh, cols],
                                mybir.AluOpType.subtract,
                            )
                        else:
                            nc.gpsimd.tensor_copy(
                                out=KS[0:64, hh, 0, cols], in_=ps[pr]
                            )
                            nc.vector.tensor_tensor(
                                KS[0:64, hh, 1, cols],
                                ps[pr],
                                KS[0:64, hh, 0, cols],
                                mybir.AluOpType.subtract,
                            )
                            nc.sync.dma_start(
                                out=KS[64:128, hh, 0, cols],
                                in_=KS[0:64, hh, 0, cols],
                            )
                for i in range(4 * j, 4 * j + 4):
                    ps = o_ps_pool.tile([128, TCH], F32, tag="ops")
                    nmm = 0
                    for w, x in ((Wh, Xh), (Wl, Xh), (Wh, Xl)):
                        for cp in range(KC // 2):
                            nmm += 1
                            nc.tensor.matmul(
                                ps[:, :VC],
                                x[:, 2 * cp : 2 * cp + 2, i * 128 : (i + 1) * 128],
                                w[:, 2 * cp : 2 * cp + 2, QKC : QKC + VC],
                                start=(nmm == 1),
                                stop=(nmm == 12),
                                perf_mode=DR,
                            )
                    nc.gpsimd.tensor_copy(
                        out=V[:, i, :, 0:D_HEAD],
                        in_=ps[:, :VC].rearrange("p (h d) -> p h d", h=HPC),
                    )

            P3 = {}

            def emit_attention(tj, phase="full"):
                if phase != "scores":
                    A = ap_.tile([128, VC // 128, TCH], BF16)
                n_si = 4 * tj + 4
                for h in range(HPC):
                    if phase != "scores":
                        U = u_ps_pool.tile([2 * D_HEAD, TCH], F32, tag="u", name="U")
                    if phase == "scores":
                        P3[h] = p3p.tile([128, NSI, TCH], BF16, tag=f"p3h{h}", name="P3")
                        P = P3[h]
                    elif phase == "pv":
                        P = P3[h]
                    else:
                        P = pp.tile([128, NSI, TCH], BF16, tag="p", name="P")
                    n_sp = n_si // 2
                    for sp in range(n_sp):
                        spair = (2 * sp, 2 * sp + 1)
                        if phase != "pv":
                            s_ps = sa_ps_pool.tile([128, 2, TCH], F32, tag="s", name="s_ps")
                            for u_, si in enumerate(spair):
                                coff = 128 * (si - 4 * tj) if si >= 4 * tj else 0
                                ncols = TCH - coff
                                qs_mov = (
                                    QS[:, h, tj * TCH + coff : (tj + 1) * TCH]
                                    .unsqueeze(1)
                                    .broadcast_to([128, 2, ncols])
                                )
                                nc.tensor.matmul(
                                    s_ps[:, u_, coff:TCH],
                                    KS[:, h, :, si * 128 : (si + 1) * 128],
                                    qs_mov,
                                    start=True,
                                    stop=True,
                                    perf_mode=DR,
                                )
                            pcoff = 128 * (spair[0] - 4 * tj) if spair[0] >= 4 * tj else 0
                            nc.scalar.activation(
                                out=P[:, 2 * sp : 2 * sp + 2, pcoff:],
                                in_=s_ps[:, :, pcoff:],
                                func=mybir.ActivationFunctionType.Exp,
                                scale=SCALE_FP8,
                            )
                            for si in spair:
                                if si >= 4 * tj:
                                    coff = 128 * (si - 4 * tj)
                                    nc.vector.tensor_tensor(
                                        P[:, si, coff : coff + 128],
                                        P[:, si, coff : coff + 128],
                                        tri,
                                        mybir.AluOpType.mult,
                                    )
                        if phase != "scores":
                            for si in spair:
                                coff = 128 * (si - 4 * tj) if si >= 4 * tj else 0
                                nc.tensor.matmul(
                                    U[:, coff:TCH],
                                    V[:, si, h, :],
                                    P[:, si, coff:TCH],
                                    start=(sp == 0 and si == spair[0]),
                                    stop=(sp == n_sp - 1 and si == spair[1]),
                                    skip_group_check=True,
                                )
                    if phase != "scores":
                        pb = 64 * (h % 2)
                        Rb = rp.tile([64, TCH], F32, tag="rbsb")
                        nc.vector.reciprocal(Rb, U[D_HEAD : 2 * D_HEAD, :])
                        a_slice = A[pb : pb + 64, h // 2, :]
                        nc.vector.tensor_tensor(
                            a_slice, U[0:D_HEAD, :], Rb, mybir.AluOpType.mult
                        )
                if phase == "scores":
                    return
                for tb in range(TCH // 128):
                    o_sb = op_.tile([128, D_MODEL], F32)
                    for n in range(D_MODEL // TCH):
                        o_ps = o_ps_pool.tile([128, TCH], F32, tag="ops")
                        for c in range(VC // 128):
                            nc.tensor.matmul(
                                o_ps,
                                A[:, c, tb * 128 : (tb + 1) * 128],
                                WP[:, c, n * TCH : (n + 1) * TCH],
                                start=(c == 0),
                                stop=(c == VC // 128 - 1),
                            )
                        eng = nc.vector if (tb + n) % 2 == 0 else nc.gpsimd
                        eng.tensor_copy(
                            out=o_sb[:, n * TCH : (n + 1) * TCH], in_=o_ps
                        )
                        nc.sync.dma_start(
                            out=out_d[
                                tj * TCH + tb * 128 : tj * TCH + (tb + 1) * 128,
                                n * TCH : (n + 1) * TCH,
                            ],
                            in_=o_sb[:, n * TCH : (n + 1) * TCH],
                        )

            cp = []
            for j in range(NTJ):
                emit_qkv_chunk(j)
                cp.append(tc.cur_priority)
            if pre3:
                for j in range(NTJ - 1):
                    off = tc.cur_priority - cp[j]
                    with tc.high_priority(offset=off):
                        emit_attention(j)
                off = tc.cur_priority - cp[NTJ - 1]
                with tc.high_priority(offset=off):
                    emit_attention(NTJ - 1, phase="scores")
                emit_attention(NTJ - 1, phase="pv")
            else:
                for j in range(NTJ):
                    off = tc.cur_priority - cp[j]
                    with tc.high_priority(offset=off):
                        emit_attention(j)
    return nc


def _build(mask_mode: str, has_qkv_bias: bool, head_pair=None, si_pair=None):
    """mask_mode: 'causal' | 'none' | 'generic'"""
    if head_pair is None:
        head_pair = os.environ.get("K_HEAD_PAIR", "0") == "1"
    if si_pair is None:
        si_pair = os.environ.get("K_SI_PAIR", "1") == "1"
    interleave = os.environ.get("K_INTERLEAVE", "0") == "1"
    _patch_drain()
    nc = bass.Bass()

    xT = nc.dram_tensor("xT", [D_MODEL, T], BF16, kind="ExternalInput")
    wqkv = nc.dram_tensor("wqkv", [D_MODEL, QKC + VC], BF16, kind="ExternalInput")
    wproj = nc.dram_tensor("wproj", [VC, D_MODEL], BF16, kind="ExternalInput")
    if mask_mode == "causal":
        tri_d = nc.dram_tensor("tri", [128, 128], BF16, kind="ExternalInput")
    if mask_mode == "generic":
        maskT_d = nc.dram_tensor("maskT", [T, T], BF16, kind="ExternalInput")
    if has_qkv_bias:
        bqk_d = nc.dram_tensor("bqk", [QKC], F32, kind="ExternalInput")
        bv_d = nc.dram_tensor("bv", [VC], F32, kind="ExternalInput")
    out_d = nc.dram_tensor("out", [T, D_MODEL], F32, kind="ExternalOutput")

    with TileContext(nc) as tc:
        with (
            tc.tile_pool(name="consts", bufs=1) as consts,
            tc.tile_pool(name="qkp", bufs=1) as qkp,
            tc.tile_pool(name="vp", bufs=1) as vp,
            tc.tile_pool(name="pp", bufs=int(os.environ.get("K_PP_BUFS", "2"))) as pp,
            tc.tile_pool(name="p3p", bufs=1) as p3p,
            tc.tile_pool(name="ap_", bufs=int(os.environ.get("K_AP_BUFS", "2"))) as ap_,
            tc.tile_pool(name="rp", bufs=int(os.environ.get("K_RP_BUFS", "2"))) as rp,
            tc.tile_pool(name="op_", bufs=int(os.environ.get("K_OSB_BUFS", "4"))) as op_,
            tc.tile_pool(name="dram_p", bufs=2, space="DRAM") as dram_p,
            tc.tile_pool(name="sa_ps", bufs=int(os.environ.get("K_SA_BUFS", str(4 // (2 if head_pair else 1) // (2 if si_pair else 1)))), space="PSUM") as sa_ps_pool,
            tc.tile_pool(name="sb_ps", bufs=(2 // (2 if si_pair else 1)), space="PSUM") as sb_ps_pool,
            tc.tile_pool(name="u_ps", bufs=int(os.environ.get("K_U_BUFS", "2")), space="PSUM") as u_ps_pool,
            tc.tile_pool(name="o_ps", bufs=int(os.environ.get("K_O_BUFS", "2")), space="PSUM") as o_ps_pool,
        ):
            # ---- load constants ----
            xT_r = xT.rearrange("(c p) t -> p c t", p=128)
            X = consts.tile([128, KC, T], BF16)
            wqkv_r = wqkv.rearrange("(c p) n -> p c n", p=128)
            W = consts.tile([128, KC, QKC + VC], BF16)
            dma_engs = [nc.sync, nc.gpsimd, nc.scalar]
            n_dma_eng = int(os.environ.get("K_DMA_ENGS", "3"))
            NQ = int(os.environ.get("K_XQ", "4"))
            for q in range(NQ):
                lo, hi = q * (T // NQ), (q + 1) * (T // NQ)
                for c in range(KC):
                    if q == 0:
                        dma_engs[c % n_dma_eng].dma_start(
                            out=W[:, c], in_=wqkv_r[:, c]
                        )
                    dma_engs[(q * KC + c + 1) % n_dma_eng].dma_start(
                        out=X[:, c, lo:hi], in_=xT_r[:, c, lo:hi]
                    )
            if mask_mode == "causal":
                tri = consts.tile([128, 128], BF16)
                nc.scalar.dma_start(out=tri, in_=tri_d[:, :])
            WP = consts.tile([128, VC // 128, D_MODEL], BF16)
            nc.sync.dma_start(out=WP, in_=wproj.rearrange("(c p) n -> p c n", p=128))
            if mask_mode == "generic":
                MT = consts.tile([128, NSI, T], BF16)
                nc.sync.dma_start(
                    out=MT, in_=maskT_d.rearrange("(si p) t -> p si t", p=128)
                )
            if has_qkv_bias:
                bqk = consts.tile([128, QKC // 128], F32)
                nc.sync.dma_start(
                    out=bqk, in_=bqk_d.rearrange("(m p) -> p m", p=128)
                )
                bv = consts.tile([128, VC // 128], F32)
                nc.sync.dma_start(out=bv, in_=bv_d.rearrange("(m p) -> p m", p=128))

            # V tile (natural layout). Each head gets 64 ones-columns
            # appended so the PV matmul (M=128, same pass cost as M=65)
            # emits the softmax sums replicated on partitions 64..127 --
            # the reciprocal+normalize then needs no partition broadcast.
            V = vp.tile([128, NSI, HPC, 2 * D_HEAD], BF16)
            nc.vector.memset(V[:, :, :, D_HEAD : 2 * D_HEAD], 1.0)
            QK = qkp.tile([128, QKC // 128, T], BF16)

            def emit_qkv_chunk(j):
                # Q^T / K^T chunk j: [qkrow, t] = sum_c W[c, qkrow] X^T[c, t]
                for m in range(QKC // 128):
                    qk_ps = o_ps_pool.tile([128, TCH], F32, tag="ops")
                    for c in range(KC):
                        nc.tensor.matmul(
                            qk_ps,
                            W[:, c, m * 128 : (m + 1) * 128],
                            X[:, c, j * TCH : (j + 1) * TCH],
                            start=(c == 0),
                            stop=(c == KC - 1),
                        )
                    if has_qkv_bias:
                        nc.scalar.activation(
                            out=QK[:, m, j * TCH : (j + 1) * TCH],
                            in_=qk_ps,
                            func=mybir.ActivationFunctionType.Identity,
                            bias=bqk[:, m : m + 1],
                        )
                    else:
                        nc.vector.tensor_copy(
                            out=QK[:, m, j * TCH : (j + 1) * TCH], in_=qk_ps
                        )
                # V rows for this chunk
                for i in range(4 * j, 4 * j + 4):
                    v_ps = o_ps_pool.tile([128, TCH], F32, tag="ops")
                    for c in range(KC):
                        nc.tensor.matmul(
                            v_ps[:, :VC],
                            X[:, c, i * 128 : (i + 1) * 128],
                            W[:, c, QKC : QKC + VC],
                            start=(c == 0),
                            stop=(c == KC - 1),
                        )
                    nc.vector.tensor_copy(
                        out=V[:, i, :, 0:D_HEAD],
                        in_=v_ps[:, :VC].rearrange("p (h d) -> p h d", h=HPC),
                    )

            pre3 = (
                os.environ.get("K_PRE3", "0") == "1" and mask_mode == "causal"
            )
            P3 = {}

            def emit_attention(tj, phase="full"):
                # phase: "full" | "scores" (S/exp/mask only, into P3 tiles)
                #        | "pv" (PV/norm/proj consuming P3 tiles)
                if phase != "scores":
                    A = ap_.tile([128, VC // 128, TCH], BF16)
                n_si = NSI if mask_mode != "causal" else 4 * tj + 4
                p_slices = 12 if pre3 else NSI
                HGRP = 2 if head_pair else 1
                SGRP = 2 if si_pair else 1
                def emit_head_group(hp):
                    heads = tuple(HGRP * hp + u for u in range(HGRP))
                    Us = {}
                    Ps = {}
                    for h in heads:
                        if phase != "scores":
                            Us[h] = u_ps_pool.tile(
                                [2 * D_HEAD, TCH], F32, tag="u", name="U"
                            )
                        if phase == "scores":
                            P3[h] = p3p.tile(
                                [128, NSI, TCH], BF16, tag=f"p3h{h}", name="P3"
                            )
                            Ps[h] = P3[h]
                        elif phase == "pv":
                            Ps[h] = P3[h]
                        else:
                            Ps[h] = pp.tile(
                                [128, p_slices, TCH], BF16, tag="p", name="P"
                            )
                    sp_order = list(range(n_si // SGRP))
                    if os.environ.get("K_SP_REV", "0") == "1":
                        sp_order = sp_order[::-1]
                    first_sp = sp_order[0]
                    last_sp = sp_order[-1]
                    for sp in sp_order:
                        spair = tuple(SGRP * sp + u for u in range(SGRP))
                        s_tiles = {}
                        for hi, h in enumerate(heads):
                            if phase == "pv":
                                break
                            pool = sa_ps_pool if hi == 0 else sb_ps_pool
                            s_ps = pool.tile([128, SGRP, TCH], F32, tag="s", name="s_ps")
                            s_tiles[h] = s_ps
                            pb = 64 * (h % 2)
                            qm = h // 2
                            km = 2 + h // 2
                            for u, si in enumerate(spair):
                                if mask_mode == "causal" and si >= 4 * tj:
                                    coff = 128 * (si - 4 * tj)
                                else:
                                    coff = 0
                                nc.tensor.matmul(
                                    s_ps[:, u, coff:TCH],
                                    QK[pb : pb + 64, km, si * 128 : (si + 1) * 128],
                                    QK[
                                        pb : pb + 64,
                                        qm,
                                        tj * TCH + coff : (tj + 1) * TCH,
                                    ],
                                    start=True,
                                    stop=True,
                                )
                        exp_split = (
                            os.environ.get("K_EXP_SPLIT", "0") == "1"
                            or tj >= int(os.environ.get("K_EXP_SPLIT_TJ", "99"))
                        )
                        for h in heads:
                            if phase == "pv":
                                break
                            # exp over the si-pair (prefixes of diagonal
                            # blocks hold garbage; never read back)
                            if exp_split:
                                for u in range(SGRP):
                                    nc.scalar.activation(
                                        out=Ps[h][:, SGRP * sp + u, :],
                                        in_=s_tiles[h][:, u, :],
                                        func=mybir.ActivationFunctionType.Exp,
                                        scale=SCALE,
                                    )
                            else:
                                if mask_mode == "causal" and spair[0] >= 4 * tj:
                                    pcoff = 128 * (spair[0] - 4 * tj)
                                else:
                                    pcoff = 0
                                nc.scalar.activation(
                                    out=Ps[h][:, SGRP * sp : SGRP * sp + SGRP, pcoff:],
                                    in_=s_tiles[h][:, :, pcoff:],
                                    func=mybir.ActivationFunctionType.Exp,
                                    scale=SCALE,
                                )
                            for si in spair:
                                if mask_mode == "causal" and si >= 4 * tj:
                                    coff = 128 * (si - 4 * tj)
                                    nc.vector.tensor_tensor(
                                        Ps[h][:, si, coff : coff + 128],
                                        Ps[h][:, si, coff : coff + 128],
                                        tri,
                                        mybir.AluOpType.mult,
                                    )
                            if mask_mode == "generic":
                                for si in spair:
                                    nc.vector.tensor_tensor(
                                        Ps[h][:, si, :],
                                        Ps[h][:, si, :],
                                        MT[:, si, tj * TCH : (tj + 1) * TCH],
                                        mybir.AluOpType.mult,
                                    )
                        for h in heads:
                            if phase == "scores":
                                break
                            for si in spair:
                                if mask_mode == "causal" and si >= 4 * tj:
                                    coff = 128 * (si - 4 * tj)
                                else:
                                    coff = 0
                                nc.tensor.matmul(
                                    Us[h][:, coff:TCH],
                                    V[:, si, h, :],
                                    Ps[h][:, si, coff:TCH],
                                    start=(sp == first_sp and si == spair[0]),
                                    stop=(sp == last_sp and si == spair[-1]),
                                    skip_group_check=True,
                                )
                    for h in heads:
                        if phase == "scores":
                            break
                        # normalize: sums sit replicated on partitions
                        # 64..127 of U; reciprocal them straight to SBUF
                        pb = 64 * (h % 2)
                        Rb_sb = rp.tile([64, TCH], F32, tag="rbsb")
                        nc.vector.reciprocal(Rb_sb, Us[h][D_HEAD : 2 * D_HEAD, :])
                        a_slice = A[pb : pb + 64, h // 2, :]
                        nc.vector.tensor_tensor(
                            a_slice, Us[h][0:D_HEAD, :], Rb_sb, mybir.AluOpType.mult
                        )
                        if has_qkv_bias:
                            nc.scalar.activation(
                                out=a_slice,
                                in_=a_slice,
                                func=mybir.ActivationFunctionType.Identity,
                                bias=bv[pb : pb + 64, h // 2 : h // 2 + 1],
                            )

                head_ilv = os.environ.get("K_HEAD_ILV", "0") == "1"
                for hp in range(HPC // HGRP):
                    if head_ilv and hp % 2 == 1:
                        off = tc.cur_priority - pair_base
                        with tc.high_priority(offset=off):
                            emit_head_group(hp)
                    else:
                        pair_base = tc.cur_priority
                        emit_head_group(hp)
                if phase == "scores":
                    return
                # proj for this t-chunk: out[t, n] = sum_c A^T[c, t] * WP[c, n]
                for tb in range(TCH // 128):
                    o_sb = op_.tile([128, D_MODEL], F32)
                    for n in range(D_MODEL // TCH):
                        o_ps = o_ps_pool.tile([128, TCH], F32, tag="ops")
                        for c in range(VC // 128):
                            nc.tensor.matmul(
                                o_ps,
                                A[:, c, tb * 128 : (tb + 1) * 128],
                                WP[:, c, n * TCH : (n + 1) * TCH],
                                start=(c == 0),
                                stop=(c == VC // 128 - 1),
                            )
                        nc.vector.tensor_copy(
                            out=o_sb[:, n * TCH : (n + 1) * TCH], in_=o_ps
                        )
                        nc.sync.dma_start(
                            out=out_d[
                                tj * TCH + tb * 128 : tj * TCH + (tb + 1) * 128,
                                n * TCH : (n + 1) * TCH,
                            ],
                            in_=o_sb[:, n * TCH : (n + 1) * TCH],
                        )

            prio_mode = os.environ.get("K_PRIO", "1") == "1"
            if interleave:
                for j in range(NTJ):
                    emit_qkv_chunk(j)
                    emit_attention(j)
            elif prio_mode:
                # emit qkv first (program order = dataflow order), but give
                # attention tj a priority window starting right after qkv
                # chunk tj, so the scheduler fills attention stalls with
                # later qkv chunks
                cp = []
                for j in range(NTJ):
                    emit_qkv_chunk(j)
                    cp.append(tc.cur_priority)
                if pre3:
                    # tj3's S/exp/mask precompute as mid-kernel filler
                    # (window right after qkv chunk 3); its PV/norm/proj
                    # run last as a dense pure-PE tail
                    for j in range(NTJ - 1):
                        off = tc.cur_priority - cp[j]
                        with tc.high_priority(offset=off):
                            emit_attention(j)
                    off = tc.cur_priority - cp[NTJ - 1]
                    with tc.high_priority(offset=off):
                        emit_attention(NTJ - 1, phase="scores")
                    emit_attention(NTJ - 1, phase="pv")
                else:
                    for j in range(NTJ):
                        off = tc.cur_priority - cp[j]
                        with tc.high_priority(offset=off):
                            emit_attention(j)
            else:
                for j in range(NTJ):
                    emit_qkv_chunk(j)
                for j in range(NTJ):
                    emit_attention(j)
    return nc


_NC_CACHE: dict = {}


def _use_fp8(mask_mode: str, has_qkv_bias: bool) -> bool:
    if os.environ.get("K_FP8", "1") != "1":
        return False
    return mask_mode == "causal" and not has_qkv_bias


def _get_nc(mask_mode: str, has_qkv_bias: bool):
    if _use_fp8(mask_mode, has_qkv_bias):
        key = "fp8"
        if key not in _NC_CACHE:
            _NC_CACHE[key] = _build_fp8()
        return _NC_CACHE[key]
    key = (mask_mode, has_qkv_bias)
    if key not in _NC_CACHE:
        _NC_CACHE[key] = _build(mask_mode, has_qkv_bias)
    return _NC_CACHE[key]


def classify_inputs(mask, qkv_b):
    m2 = np.asarray(mask).reshape(T, T)
    if np.array_equal(m2 != 0, np.tril(np.ones((T, T), dtype=bool))):
        mask_mode = "causal"
    elif np.all(m2 != 0):
        mask_mode = "none"
    else:
        mask_mode = "generic"
    has_qkv_bias = bool(np.any(np.asarray(qkv_b) != 0.0))
    return mask_mode, has_qkv_bias


def prepare_in_maps(x, mask, qkv_w, qkv_b, proj_w, proj_b):
    x = np.asarray(x, dtype=np.float32)
    qkv_w = np.asarray(qkv_w, dtype=np.float32)
    qkv_b = np.asarray(qkv_b, dtype=np.float32)
    proj_w = np.asarray(proj_w, dtype=np.float32)
    mask_mode, has_qkv_bias = classify_inputs(mask, qkv_b)
    m2 = np.asarray(mask).reshape(T, T)

    tri_np = np.triu(np.ones((128, 128))).astype(NP_BF16)
    in_maps = []
    for b in range(B):
        xT_b = np.ascontiguousarray(x[b].T).astype(NP_BF16)
        for g in range(G):
            qs = qkv_w[:, g * VC : (g + 1) * VC]
            ks = qkv_w[:, D_MODEL + g * VC : D_MODEL + (g + 1) * VC]
            vs = qkv_w[:, 2 * D_MODEL + g * VC : 2 * D_MODEL + (g + 1) * VC]
            im = {
                "xT": xT_b,
                "wqkv": np.ascontiguousarray(
                    np.concatenate([qs, ks, vs], axis=1)
                ).astype(NP_BF16),
                "wproj": np.ascontiguousarray(
                    proj_w[g * VC : (g + 1) * VC, :]
                ).astype(NP_BF16),
            }
            if mask_mode == "causal":
                im["tri"] = tri_np
            if mask_mode == "generic":
                im["maskT"] = np.ascontiguousarray(
                    (m2 != 0).T.astype(NP_BF16)
                )
            if has_qkv_bias:
                im["bqk"] = np.ascontiguousarray(
                    np.concatenate(
                        [qkv_b[g * VC : (g + 1) * VC],
                         qkv_b[D_MODEL + g * VC : D_MODEL + (g + 1) * VC]]
                    )
                ).astype(np.float32)
                im["bv"] = np.ascontiguousarray(
                    qkv_b[2 * D_MODEL + g * VC : 2 * D_MODEL + (g + 1) * VC]
                ).astype(np.float32)
            in_maps.append(im)
    return in_maps


def prepare_in_maps_fp8(x, qkv_w, proj_w):
    x = np.asarray(x, dtype=np.float32)
    qkv_w = np.asarray(qkv_w, dtype=np.float32) * S_W
    proj_w = np.asarray(proj_w, dtype=np.float32) / S_W
    tri_np = np.triu(np.ones((128, 128))).astype(NP_BF16)
    ones_np = np.ones((128, NSI * HPC * D_HEAD), dtype=NP_BF16)
    zeros_np = np.zeros((64, HPC * T), dtype=NP_F8)
    in_maps = []
    for b in range(B):
        xT = np.ascontiguousarray(x[b].T)
        xh = xT.astype(NP_F8)
        xl = (xT - xh.astype(np.float32)).astype(NP_F8)
        for g in range(G):
            qs = qkv_w[:, g * VC : (g + 1) * VC]
            ks = qkv_w[:, D_MODEL + g * VC : D_MODEL + (g + 1) * VC]
            vs = qkv_w[:, 2 * D_MODEL + g * VC : 2 * D_MODEL + (g + 1) * VC]
            w = np.ascontiguousarray(np.concatenate([qs, ks, vs], axis=1))
            wh = w.astype(NP_F8)
            wl = (w - wh.astype(np.float32)).astype(NP_F8)
            in_maps.append(
                {
                    "xh": xh,
                    "xl": xl,
                    "wh": wh,
                    "wl": wl,
                    "wproj": np.ascontiguousarray(
                        proj_w[g * VC : (g + 1) * VC, :]
                    ).astype(NP_BF16),
                    "tri": tri_np,
                    "ones": ones_np,
                    "zeros": zeros_np,
                }
            )
    return in_maps


def kernel(x, mask, qkv_w, qkv_b, proj_w, proj_b):
    proj_b = np.asarray(proj_b, dtype=np.float32)
    mask_mode, has_qkv_bias = classify_inputs(mask, qkv_b)
    nc = _get_nc(mask_mode, has_qkv_bias)
    if _use_fp8(mask_mode, has_qkv_bias):
        in_maps = prepare_in_maps_fp8(x, qkv_w, proj_w)
    else:
        in_maps = prepare_in_maps(x, mask, qkv_w, qkv_b, proj_w, proj_b)

    trace = bool(os.environ.get("KERNEL_TRACE"))
    res = run_bass_kernel_spmd(
        nc, in_maps, core_ids=list(range(B * G)), trace=trace
    )
    globals()["LAST_RESULT"] = res
    outs = [np.asarray(r["out"], dtype=np.float32) for r in res.results]

    final = np.empty((B, T, D_MODEL), dtype=np.float32)
    for b in range(B):
        acc = outs[b * G].copy()
        for g in range(1, G):
            acc += outs[b * G + g]
        final[b] = acc + proj_b[None, :]
    return final



# revision 16
# speedup vs baseline: 1.7544x; 1.0580x over previous
"""Causal self-attention Trainium2 kernel (B=2, T=2048, D=1024, 16 heads).

Sharding: 8 cores = 2 batches x 4 head-groups (4 heads each).
Per core: column-parallel qkv, local attention, row-parallel proj producing a
partial output; host sums the 4 partials per batch and adds proj bias.
"""

import json
import math
import os

import numpy as np
import ml_dtypes

import bass_rust
import concourse.bass as bass
import concourse.bass2jax as bass2jax
import concourse.bass_utils as bass_utils
import concourse.mybir as mybir
import concourse.tile as tile
from concourse.tile import TileContext
from concourse.vector_clock import ScopedClock
from concourse.bass_utils import run_bass_kernel_spmd

BF16 = mybir.dt.bfloat16
F32 = mybir.dt.float32
NP_BF16 = ml_dtypes.bfloat16

D_MODEL = 1024
N_HEAD = 16
D_HEAD = 64
B = 2
T = 2048
G = 4                    # head-groups (tensor parallel)
HPC = N_HEAD // G        # heads per core = 4
QKC = 2 * HPC * D_HEAD   # q+k rows per core = 512
VC = HPC * D_HEAD        # v cols per core = 256
TCH = 512                # t-chunk (matmul moving free dim)
NTJ = T // TCH           # 4 t-chunks
NSI = T // 128           # 16 s-blocks
KC = D_MODEL // 128      # 8 contraction chunks over d_model
SCALE = 1.0 / math.sqrt(D_HEAD)


def _split_multi_waits(bir_json: bytes) -> bytes:
    """The walrus build in this container accepts at most one sync-wait
    command per instruction. Split instructions with N>1 waits into N-1
    preceding single-wait NoOps on the same engine (sequential waits AND
    together, so semantics are unchanged)."""
    bir = json.loads(bir_json)
    ctr = 0
    changed = False
    for func in bir.get("functions", []):
        for blk in func.get("blocks", []):
            out = []
            for inst in blk.get("instructions", []):
                si = inst.get("sync_info")
                waits = (si or {}).get("on_wait") or []
                if len(waits) > 1:
                    changed = True
                    for w in waits[:-1]:
                        ctr += 1
                        out.append(
                            {
                                "debug": inst.get("debug", 0),
                                "engine": inst.get("engine"),
                                "ins": [],
                                "name": f"WSPLIT-{ctr}",
                                "opcode": "NoOp",
                                "outs": [],
                                "sync_info": {"on_update": [], "on_wait": [w]},
                            }
                        )
                    si["on_wait"] = [waits[-1]]
                out.append(inst)
            blk["instructions"] = out
    if not changed:
        return bir_json
    return json.dumps(bir).encode()


_orig_compile_bir_kernel = bass_utils.compile_bir_kernel.__wrapped__ if hasattr(
    bass_utils.compile_bir_kernel, "__wrapped__"
) else bass_utils.compile_bir_kernel


def _patched_compile_bir_kernel(bir_json, tmpdir, neff_name="file.neff"):
    return _orig_compile_bir_kernel(_split_multi_waits(bir_json), tmpdir, neff_name)


def _patch_drain():
    """The walrus build in this container rejects >1 sync-wait command per
    instruction. Patch the compile path to split waits, and the TileContext
    terminal drain to emit single-wait SP nops."""
    if getattr(TileContext, "_drain_patched", False):
        return
    bass_utils.compile_bir_kernel = _patched_compile_bir_kernel
    bass2jax.compile_bir_kernel = _patched_compile_bir_kernel

    def _drain_and_barrier(self, tick_clock, wait_clock):
        nc = self.nc
        probe = nc.sync.nop()
        wait_clock.add_sem_waits(
            probe.ins, ScopedClock({None: tick_clock.global_clock})
        )
        si = probe.ins.sync_info
        waits = list(si.on_wait) if si is not None else []
        if si is not None:
            si.on_wait = waits[:1]
            probe.ins.sync_info = si
        for w in waits[1:]:
            n = nc.sync.nop()
            n.ins.sync_info = bass_rust.SyncInfo(on_wait=[w], on_update=[])
        nc.sync.drain()
        nc.all_engine_barrier()
        popped = nc._tile_sem_poison_stack.pop()
        assert popped is self._sem_poison
        nc.clear_and_free_semaphores(list(self.sems.allocated().values()))
        nc.all_engine_barrier()

    TileContext._drain_and_barrier = _drain_and_barrier
    TileContext._drain_patched = True


F8 = mybir.dt.float8e4
NP_F8 = ml_dtypes.float8_e4m3
S_W = 16.0                      # qkv weight prescale (fp8 subnormal avoidance)
SCALE_FP8 = SCALE / (S_W * S_W)  # folded into the exp activation
DR = mybir.MatmulPerfMode.DoubleRow


def _build_fp8():
    """Causal, no-bias path with fp8 DoubleRow matmuls.

    QKV: 3-term hi/lo split (hi*hi + lo*hi + hi*lo), DoubleRow pairs over
    adjacent d_model chunks -> 0.75x the bf16 matmul time.
    Scores: contraction stacked as [hi(64); lo(64)] on partitions.
      stationary KS pair0 = [Khi; Khi], pair1 = [Klo; 0]
      moving    QS = [Qhi; Qlo], broadcast along the pair dim
      -> one DR matmul per s-block = 2x the bf16 rate, 3-term accurate.
    PV / proj stay bf16.
    """
    _patch_drain()
    nc = bass.Bass()

    xh_d = nc.dram_tensor("xh", [D_MODEL, T], F8, kind="ExternalInput")
    xl_d = nc.dram_tensor("xl", [D_MODEL, T], F8, kind="ExternalInput")
    wh_d = nc.dram_tensor("wh", [D_MODEL, QKC + VC], F8, kind="ExternalInput")
    wl_d = nc.dram_tensor("wl", [D_MODEL, QKC + VC], F8, kind="ExternalInput")
    wp_d = nc.dram_tensor("wproj", [VC, D_MODEL], BF16, kind="ExternalInput")
    tri_d = nc.dram_tensor("tri", [128, 128], BF16, kind="ExternalInput")
    zeros_d = nc.dram_tensor("zeros", [64, HPC * T], F8, kind="ExternalInput")
    out_d = nc.dram_tensor("out", [T, D_MODEL], F32, kind="ExternalOutput")

    pre3 = os.environ.get("K_PRE3", "1") == "1"

    with TileContext(nc) as tc:
        with (
            tc.tile_pool(name="consts", bufs=1) as consts,
            tc.tile_pool(name="qkp", bufs=1) as qkp,
            tc.tile_pool(name="vp", bufs=1) as vp,
            tc.tile_pool(name="pp", bufs=int(os.environ.get("K_PP_BUFS", "2"))) as pp,
            tc.tile_pool(name="p3p", bufs=1) as p3p,
            tc.tile_pool(name="ap_", bufs=int(os.environ.get("K_AP_BUFS", "2"))) as ap_,
            tc.tile_pool(name="rp", bufs=int(os.environ.get("K_RP_BUFS", "2"))) as rp,
            tc.tile_pool(name="op_", bufs=int(os.environ.get("K_OSB_BUFS", "3"))) as op_,
            tc.tile_pool(name="sa_ps", bufs=int(os.environ.get("K_SA_BUFS", "2")), space="PSUM") as sa_ps_pool,
            tc.tile_pool(name="u_ps", bufs=int(os.environ.get("K_U_BUFS", "2")), space="PSUM") as u_ps_pool,
            tc.tile_pool(name="o_ps", bufs=int(os.environ.get("K_O_BUFS", "2")), space="PSUM") as o_ps_pool,
        ):
            # ---- constants ----
            Xh = consts.tile([128, KC, T], F8)
            Xl = consts.tile([128, KC, T], F8)
            Wh = consts.tile([128, KC, QKC + VC], F8)
            Wl = consts.tile([128, KC, QKC + VC], F8)
            WP = consts.tile([128, VC // 128, D_MODEL], BF16)
            tri = consts.tile([128, 128], BF16)
            # QS: per head [Qhi(p0:64); Qlo(p64:128)] over t
            QS = qkp.tile([128, HPC, T], F8)
            # KS: per head, pair0 = [Khi; Khi], pair1 = [Klo; 0]
            KS = qkp.tile([128, HPC, 2, T], F8)
            V = vp.tile([128, NSI, HPC, 2 * D_HEAD], BF16)

            xh_r = xh_d.rearrange("(c p) t -> p c t", p=128)
            xl_r = xl_d.rearrange("(c p) t -> p c t", p=128)
            wh_r = wh_d.rearrange("(c p) n -> p c n", p=128)
            wl_r = wl_d.rearrange("(c p) n -> p c n", p=128)
            NQ = int(os.environ.get("K_XQ", "4"))
            nc.sync.dma_start(out=Wh, in_=wh_r)
            for q in range(NQ):
                lo, hi = q * (T // NQ), (q + 1) * (T // NQ)
                nc.sync.dma_start(out=Xh[:, :, lo:hi], in_=xh_r[:, :, lo:hi])
                if q == 0:
                    nc.sync.dma_start(out=Wl, in_=wl_r)
                nc.sync.dma_start(out=Xl[:, :, lo:hi], in_=xl_r[:, :, lo:hi])
                if q == 0:
                    # extras issued after the critical first-quarter loads
                    # (the DMA resource is serial in the cost model)
                    nc.sync.dma_start(out=tri, in_=tri_d[:, :])
                    nc.sync.dma_start(
                        out=KS[64:128, :, 1, :],
                        in_=zeros_d.rearrange("p (h t) -> p h t", h=HPC),
                    )
                if q == 1:
                    nc.sync.dma_start(
                        out=WP, in_=wp_d.rearrange("(c p) n -> p c n", p=128)
                    )

            def emit_qkv_chunk(j):
                cols = slice(j * TCH, (j + 1) * TCH)
                # ones columns for this chunk's V blocks (PV softmax-sum trick)
                nc.gpsimd.memset(
                    V[:, 4 * j : 4 * j + 4, :, D_HEAD : 2 * D_HEAD], 1.0
                )
                # order q0, k0, q1, k1 so head 0/1 attention unblocks first
                for m in (0, 2, 1, 3):
                    ps = o_ps_pool.tile([128, TCH], F32, tag="ops")
                    nmm = 0
                    for w, x in ((Wh, Xh), (Wl, Xh), (Wh, Xl)):
                        for cp in range(KC // 2):
                            nmm += 1
                            nc.tensor.matmul(
                                ps,
                                w[:, 2 * cp : 2 * cp + 2, m * 128 : (m + 1) * 128],
                                x[:, 2 * cp : 2 * cp + 2, cols],
                                start=(nmm == 1),
                                stop=(nmm == 12),
                                perf_mode=DR,
                            )
                    if m < 2:
                        heads = (2 * m, 2 * m + 1)
                    else:
                        heads = (2 * (m - 2), 2 * (m - 2) + 1)
                    for u, hh in enumerate(heads):
                        pr = slice(64 * u, 64 * u + 64)
                        if m < 2:
                            nc.gpsimd.tensor_copy(
                                out=QS[0:64, hh, cols], in_=ps[pr]
                            )
                            nc.vector.tensor_tensor(
                                QS[64:128, hh, cols],
                                ps[pr],
                                QS[0:64, hh, cols],
                                mybir.AluOpType.subtract,
                            )
                        else:
                            nc.gpsimd.tensor_copy(
                                out=KS[0:64, hh, 0, cols], in_=ps[pr]
                            )
                            nc.vector.tensor_tensor(
                                KS[0:64, hh, 1, cols],
                                ps[pr],
                                KS[0:64, hh, 0, cols],
                                mybir.AluOpType.subtract,
                            )
                            nc.sync.dma_start(
                                out=KS[64:128, hh, 0, cols],
                                in_=KS[0:64, hh, 0, cols],
                            )
                for i in range(4 * j, 4 * j + 4):
                    ps = o_ps_pool.tile([128, TCH], F32, tag="ops")
                    nmm = 0
                    for w, x in ((Wh, Xh), (Wl, Xh), (Wh, Xl)):
                        for cp in range(KC // 2):
                            nmm += 1
                            nc.tensor.matmul(
                                ps[:, :VC],
                                x[:, 2 * cp : 2 * cp + 2, i * 128 : (i + 1) * 128],
                                w[:, 2 * cp : 2 * cp + 2, QKC : QKC + VC],
                                start=(nmm == 1),
                                stop=(nmm == 12),
                                perf_mode=DR,
                            )
                    nc.gpsimd.tensor_copy(
                        out=V[:, i, :, 0:D_HEAD],
                        in_=ps[:, :VC].rearrange("p (h d) -> p h d", h=HPC),
                    )

            P3 = {}

            def emit_attention(tj, phase="full"):
                if phase != "scores":
                    A = ap_.tile([128, VC // 128, TCH], BF16)
                n_si = 4 * tj + 4
                for h in range(HPC):
                    if phase != "scores":
                        U = u_ps_pool.tile([2 * D_HEAD, TCH], F32, tag="u", name="U")
                    if phase == "scores":
                        P3[h] = p3p.tile([128, NSI, TCH], BF16, tag=f"p3h{h}", name="P3")
                        P = P3[h]
                    elif phase == "pv":
                        P = P3[h]
                    else:
                        P = pp.tile([128, NSI, TCH], BF16, tag="p", name="P")
                    n_sp = n_si // 2
                    for sp in range(n_sp):
                        spair = (2 * sp, 2 * sp + 1)
                        if phase != "pv":
                            s_ps = sa_ps_pool.tile([128, 2, TCH], F32, tag="s", name="s_ps")
                            for u_, si in enumerate(spair):
                                coff = 128 * (si - 4 * tj) if si >= 4 * tj else 0
                                ncols = TCH - coff
                                qs_mov = (
                                    QS[:, h, tj * TCH + coff : (tj + 1) * TCH]
                                    .unsqueeze(1)
                                    .broadcast_to([128, 2, ncols])
                                )
                                nc.tensor.matmul(
                                    s_ps[:, u_, coff:TCH],
                                    KS[:, h, :, si * 128 : (si + 1) * 128],
                                    qs_mov,
                                    start=True,
                                    stop=True,
                                    perf_mode=DR,
                                )
                            pcoff = 128 * (spair[0] - 4 * tj) if spair[0] >= 4 * tj else 0
                            nc.scalar.activation(
                                out=P[:, 2 * sp : 2 * sp + 2, pcoff:],
                                in_=s_ps[:, :, pcoff:],
                                func=mybir.ActivationFunctionType.Exp,
                                scale=SCALE_FP8,
                            )
                            for si in spair:
                                if si >= 4 * tj:
                                    coff = 128 * (si - 4 * tj)
                                    nc.vector.tensor_tensor(
                                        P[:, si, coff : coff + 128],
                                        P[:, si, coff : coff + 128],
                                        tri,
                                        mybir.AluOpType.mult,
                                    )
                        if phase != "scores":
                            for si in spair:
                                coff = 128 * (si - 4 * tj) if si >= 4 * tj else 0
                                nc.tensor.matmul(
                                    U[:, coff:TCH],
                                    V[:, si, h, :],
                                    P[:, si, coff:TCH],
                                    start=(sp == 0 and si == spair[0]),
                                    stop=(sp == n_sp - 1 and si == spair[1]),
                                    skip_group_check=True,
                                )
                    if phase != "scores":
                        pb = 64 * (h % 2)
                        Rb = rp.tile([64, TCH], F32, tag="rbsb")
                        nc.vector.reciprocal(Rb, U[D_HEAD : 2 * D_HEAD, :])
                        a_slice = A[pb : pb + 64, h // 2, :]
                        nc.vector.tensor_tensor(
                            a_slice, U[0:D_HEAD, :], Rb, mybir.AluOpType.mult
                        )
                if phase == "scores":
                    return
                for tb in range(TCH // 128):
                    o_sb = op_.tile([128, D_MODEL], F32)
                    for n in range(D_MODEL // TCH):
                        o_ps = o_ps_pool.tile([128, TCH], F32, tag="ops")
                        for c in range(VC // 128):
                            nc.tensor.matmul(
                                o_ps,
                                A[:, c, tb * 128 : (tb + 1) * 128],
                                WP[:, c, n * TCH : (n + 1) * TCH],
                                start=(c == 0),
                                stop=(c == VC // 128 - 1),
                            )
                        eng = nc.vector if (tb + n) % 2 == 0 else nc.gpsimd
                        eng.tensor_copy(
                            out=o_sb[:, n * TCH : (n + 1) * TCH], in_=o_ps
                        )
                        nc.sync.dma_start(
                            out=out_d[
                                tj * TCH + tb * 128 : tj * TCH + (tb + 1) * 128,
                                n * TCH : (n + 1) * TCH,
                            ],
                            in_=o_sb[:, n * TCH : (n + 1) * TCH],
                        )

            cp = []
            for j in range(NTJ):
                emit_qkv_chunk(j)
                cp.append(tc.cur_priority)
            if pre3:
                for j in range(NTJ - 1):
                    off = tc.cur_priority - cp[j]
                    with tc.high_priority(offset=off):
                        emit_attention(j)
                off = tc.cur_priority - cp[NTJ - 1]
                with tc.high_priority(offset=off):
                    emit_attention(NTJ - 1, phase="scores")
                emit_attention(NTJ - 1, phase="pv")
            else:
                for j in range(NTJ):
                    off = tc.cur_priority - cp[j]
                    with tc.high_priority(offset=off):
                        emit_attention(j)
    return nc


def _build(mask_mode: str, has_qkv_bias: bool, head_pair=None, si_pair=None):
    """mask_mode: 'causal' | 'none' | 'generic'"""
    if head_pair is None:
        head_pair = os.environ.get("K_HEAD_PAIR", "0") == "1"
    if si_pair is None:
        si_pair = os.environ.get("K_SI_PAIR", "1") == "1"
    interleave = os.environ.get("K_INTERLEAVE", "0") == "1"
    _patch_drain()
    nc = bass.Bass()

    xT = nc.dram_tensor("xT", [D_MODEL, T], BF16, kind="ExternalInput")
    wqkv = nc.dram_tensor("wqkv", [D_MODEL, QKC + VC], BF16, kind="ExternalInput")
    wproj = nc.dram_tensor("wproj", [VC, D_MODEL], BF16, kind="ExternalInput")
    if mask_mode == "causal":
        tri_d = nc.dram_tensor("tri", [128, 128], BF16, kind="ExternalInput")
    if mask_mode == "generic":
        maskT_d = nc.dram_tensor("maskT", [T, T], BF16, kind="ExternalInput")
    if has_qkv_bias:
        bqk_d = nc.dram_tensor("bqk", [QKC], F32, kind="ExternalInput")
        bv_d = nc.dram_tensor("bv", [VC], F32, kind="ExternalInput")
    out_d = nc.dram_tensor("out", [T, D_MODEL], F32, kind="ExternalOutput")

    with TileContext(nc) as tc:
        with (
            tc.tile_pool(name="consts", bufs=1) as consts,
            tc.tile_pool(name="qkp", bufs=1) as qkp,
            tc.tile_pool(name="vp", bufs=1) as vp,
            tc.tile_pool(name="pp", bufs=int(os.environ.get("K_PP_BUFS", "2"))) as pp,
            tc.tile_pool(name="p3p", bufs=1) as p3p,
            tc.tile_pool(name="ap_", bufs=int(os.environ.get("K_AP_BUFS", "2"))) as ap_,
            tc.tile_pool(name="rp", bufs=int(os.environ.get("K_RP_BUFS", "2"))) as rp,
            tc.tile_pool(name="op_", bufs=int(os.environ.get("K_OSB_BUFS", "4"))) as op_,
            tc.tile_pool(name="dram_p", bufs=2, space="DRAM") as dram_p,
            tc.tile_pool(name="sa_ps", bufs=int(os.environ.get("K_SA_BUFS", str(4 // (2 if head_pair else 1) // (2 if si_pair else 1)))), space="PSUM") as sa_ps_pool,
            tc.tile_pool(name="sb_ps", bufs=(2 // (2 if si_pair else 1)), space="PSUM") as sb_ps_pool,
            tc.tile_pool(name="u_ps", bufs=int(os.environ.get("K_U_BUFS", "2")), space="PSUM") as u_ps_pool,
            tc.tile_pool(name="o_ps", bufs=int(os.environ.get("K_O_BUFS", "2")), space="PSUM") as o_ps_pool,
        ):
            # ---- load constants ----
            xT_r = xT.rearrange("(c p) t -> p c t", p=128)
            X = consts.tile([128, KC, T], BF16)
            wqkv_r = wqkv.rearrange("(c p) n -> p c n", p=128)
            W = consts.tile([128, KC, QKC + VC], BF16)
            dma_engs = [nc.sync, nc.gpsimd, nc.scalar]
            n_dma_eng = int(os.environ.get("K_DMA_ENGS", "3"))
            NQ = int(os.environ.get("K_XQ", "4"))
            for q in range(NQ):
                lo, hi = q * (T // NQ), (q + 1) * (T // NQ)
                for c in range(KC):
                    if q == 0:
                        dma_engs[c % n_dma_eng].dma_start(
                            out=W[:, c], in_=wqkv_r[:, c]
                        )
                    dma_engs[(q * KC + c + 1) % n_dma_eng].dma_start(
                        out=X[:, c, lo:hi], in_=xT_r[:, c, lo:hi]
                    )
            if mask_mode == "causal":
                tri = consts.tile([128, 128], BF16)
                nc.scalar.dma_start(out=tri, in_=tri_d[:, :])
            WP = consts.tile([128, VC // 128, D_MODEL], BF16)
            nc.sync.dma_start(out=WP, in_=wproj.rearrange("(c p) n -> p c n", p=128))
            if mask_mode == "generic":
                MT = consts.tile([128, NSI, T], BF16)
                nc.sync.dma_start(
                    out=MT, in_=maskT_d.rearrange("(si p) t -> p si t", p=128)
                )
            if has_qkv_bias:
                bqk = consts.tile([128, QKC // 128], F32)
                nc.sync.dma_start(
                    out=bqk, in_=bqk_d.rearrange("(m p) -> p m", p=128)
                )
                bv = consts.tile([128, VC // 128], F32)
                nc.sync.dma_start(out=bv, in_=bv_d.rearrange("(m p) -> p m", p=128))

            # V tile (natural layout). Each head gets 64 ones-columns
            # appended so the PV matmul (M=128, same pass cost as M=65)
            # emits the softmax sums replicated on partitions 64..127 --
            # the reciprocal+normalize then needs no partition broadcast.
            V = vp.tile([128, NSI, HPC, 2 * D_HEAD], BF16)
            nc.vector.memset(V[:, :, :, D_HEAD : 2 * D_HEAD], 1.0)
            QK = qkp.tile([128, QKC // 128, T], BF16)

            def emit_qkv_chunk(j):
                # Q^T / K^T chunk j: [qkrow, t] = sum_c W[c, qkrow] X^T[c, t]
                for m in range(QKC // 128):
                    qk_ps = o_ps_pool.tile([128, TCH], F32, tag="ops")
                    for c in range(KC):
                        nc.tensor.matmul(
                            qk_ps,
                            W[:, c, m * 128 : (m + 1) * 128],
                            X[:, c, j * TCH : (j + 1) * TCH],
                            start=(c == 0),
                            stop=(c == KC - 1),
                        )
                    if has_qkv_bias:
                        nc.scalar.activation(
                            out=QK[:, m, j * TCH : (j + 1) * TCH],
                            in_=qk_ps,
                            func=mybir.ActivationFunctionType.Identity,
                            bias=bqk[:, m : m + 1],
                        )
                    else:
                        nc.vector.tensor_copy(
                            out=QK[:, m, j * TCH : (j + 1) * TCH], in_=qk_ps
                        )
                # V rows for this chunk
                for i in range(4 * j, 4 * j + 4):
                    v_ps = o_ps_pool.tile([128, TCH], F32, tag="ops")
                    for c in range(KC):
                        nc.tensor.matmul(
                            v_ps[:, :VC],
                            X[:, c, i * 128 : (i + 1) * 128],
                            W[:, c, QKC : QKC + VC],
                            start=(c == 0),
                            stop=(c == KC - 1),
                        )
                    nc.vector.tensor_copy(
                        out=V[:, i, :, 0:D_HEAD],
                        in_=v_ps[:, :VC].rearrange("p (h d) -> p h d", h=HPC),
                    )

            pre3 = (
                os.environ.get("K_PRE3", "0") == "1" and mask_mode == "causal"
            )
            P3 = {}

            def emit_attention(tj, phase="full"):
                # phase: "full" | "scores" (S/exp/mask only, into P3 tiles)
                #        | "pv" (PV/norm/proj consuming P3 tiles)
                if phase != "scores":
                    A = ap_.tile([128, VC // 128, TCH], BF16)
                n_si = NSI if mask_mode != "causal" else 4 * tj + 4
                p_slices = 12 if pre3 else NSI
                HGRP = 2 if head_pair else 1
                SGRP = 2 if si_pair else 1
                def emit_head_group(hp):
                    heads = tuple(HGRP * hp + u for u in range(HGRP))
                    Us = {}
                    Ps = {}
                    for h in heads:
                        if phase != "scores":
                            Us[h] = u_ps_pool.tile(
                                [2 * D_HEAD, TCH], F32, tag="u", name="U"
                            )
                        if phase == "scores":
                            P3[h] = p3p.tile(
                                [128, NSI, TCH], BF16, tag=f"p3h{h}", name="P3"
                            )
                            Ps[h] = P3[h]
                        elif phase == "pv":
                            Ps[h] = P3[h]
                        else:
                            Ps[h] = pp.tile(
                                [128, p_slices, TCH], BF16, tag="p", name="P"
                            )
                    sp_order = list(range(n_si // SGRP))
                    if os.environ.get("K_SP_REV", "0") == "1":
                        sp_order = sp_order[::-1]
                    first_sp = sp_order[0]
                    last_sp = sp_order[-1]
                    for sp in sp_order:
                        spair = tuple(SGRP * sp + u for u in range(SGRP))
                        s_tiles = {}
                        for hi, h in enumerate(heads):
                            if phase == "pv":
                                break
                            pool = sa_ps_pool if hi == 0 else sb_ps_pool
                            s_ps = pool.tile([128, SGRP, TCH], F32, tag="s", name="s_ps")
                            s_tiles[h] = s_ps
                            pb = 64 * (h % 2)
                            qm = h // 2
                            km = 2 + h // 2
                            for u, si in enumerate(spair):
                                if mask_mode == "causal" and si >= 4 * tj:
                                    coff = 128 * (si - 4 * tj)
                                else:
                                    coff = 0
                                nc.tensor.matmul(
                                    s_ps[:, u, coff:TCH],
                                    QK[pb : pb + 64, km, si * 128 : (si + 1) * 128],
                                    QK[
                                        pb : pb + 64,
                                        qm,
                                        tj * TCH + coff : (tj + 1) * TCH,
                                    ],
                                    start=True,
                                    stop=True,
                                )
                        exp_split = (
                            os.environ.get("K_EXP_SPLIT", "0") == "1"
                            or tj >= int(os.environ.get("K_EXP_SPLIT_TJ", "99"))
                        )
                        for h in heads:
                            if phase == "pv":
                                break
                            # exp over the si-pair (prefixes of diagonal
                            # blocks hold garbage; never read back)
                            if exp_split:
                                for u in range(SGRP):
                                    nc.scalar.activation(
                                        out=Ps[h][:, SGRP * sp + u, :],
                                        in_=s_tiles[h][:, u, :],
                                        func=mybir.ActivationFunctionType.Exp,
                                        scale=SCALE,
                                    )
                            else:
                                if mask_mode == "causal" and spair[0] >= 4 * tj:
                                    pcoff = 128 * (spair[0] - 4 * tj)
                                else:
                                    pcoff = 0
                                nc.scalar.activation(
                                    out=Ps[h][:, SGRP * sp : SGRP * sp + SGRP, pcoff:],
                                    in_=s_tiles[h][:, :, pcoff:],
                                    func=mybir.ActivationFunctionType.Exp,
                                    scale=SCALE,
                                )
                            for si in spair:
                                if mask_mode == "causal" and si >= 4 * tj:
                                    coff = 128 * (si - 4 * tj)
                                    nc.vector.tensor_tensor(
                                        Ps[h][:, si, coff : coff + 128],
                                        Ps[h][:, si, coff : coff + 128],
                                        tri,
                                        mybir.AluOpType.mult,
                                    )
                            if mask_mode == "generic":
                                for si in spair:
                                    nc.vector.tensor_tensor(
                                        Ps[h][:, si, :],
                                        Ps[h][:, si, :],
                                        MT[:, si, tj * TCH : (tj + 1) * TCH],
                                        mybir.AluOpType.mult,
                                    )
                        for h in heads:
                            if phase == "scores":
                                break
                            for si in spair:
                                if mask_mode == "causal" and si >= 4 * tj:
                                    coff = 128 * (si - 4 * tj)
                                else:
                                    coff = 0
                                nc.tensor.matmul(
                                    Us[h][:, coff:TCH],
                                    V[:, si, h, :],
                                    Ps[h][:, si, coff:TCH],
                                    start=(sp == first_sp and si == spair[0]),
                                    stop=(sp == last_sp and si == spair[-1]),
                                    skip_group_check=True,
                                )
                    for h in heads:
                        if phase == "scores":
                            break
                        # normalize: sums sit replicated on partitions
                        # 64..127 of U; reciprocal them straight to SBUF
                        pb = 64 * (h % 2)
                        Rb_sb = rp.tile([64, TCH], F32, tag="rbsb")
                        nc.vector.reciprocal(Rb_sb, Us[h][D_HEAD : 2 * D_HEAD, :])
                        a_slice = A[pb : pb + 64, h // 2, :]
                        nc.vector.tensor_tensor(
                            a_slice, Us[h][0:D_HEAD, :], Rb_sb, mybir.AluOpType.mult
                        )
                        if has_qkv_bias:
                            nc.scalar.activation(
                                out=a_slice,
                                in_=a_slice,
                                func=mybir.ActivationFunctionType.Identity,
                                bias=bv[pb : pb + 64, h // 2 : h // 2 + 1],
                            )

                head_ilv = os.environ.get("K_HEAD_ILV", "0") == "1"
                for hp in range(HPC // HGRP):
                    if head_ilv and hp % 2 == 1:
                        off = tc.cur_priority - pair_base
                        with tc.high_priority(offset=off):
                            emit_head_group(hp)
                    else:
                        pair_base = tc.cur_priority
                        emit_head_group(hp)
                if phase == "scores":
                    return
                # proj for this t-chunk: out[t, n] = sum_c A^T[c, t] * WP[c, n]
                for tb in range(TCH // 128):
                    o_sb = op_.tile([128, D_MODEL], F32)
                    for n in range(D_MODEL // TCH):
                        o_ps = o_ps_pool.tile([128, TCH], F32, tag="ops")
                        for c in range(VC // 128):
                            nc.tensor.matmul(
                                o_ps,
                                A[:, c, tb * 128 : (tb + 1) * 128],
                                WP[:, c, n * TCH : (n + 1) * TCH],
                                start=(c == 0),
                                stop=(c == VC // 128 - 1),
                            )
                        nc.vector.tensor_copy(
                            out=o_sb[:, n * TCH : (n + 1) * TCH], in_=o_ps
                        )
                        nc.sync.dma_start(
                            out=out_d[
                                tj * TCH + tb * 128 : tj * TCH + (tb + 1) * 128,
                                n * TCH : (n + 1) * TCH,
                            ],
                            in_=o_sb[:, n * TCH : (n + 1) * TCH],
                        )

            prio_mode = os.environ.get("K_PRIO", "1") == "1"
            if interleave:
                for j in range(NTJ):
                    emit_qkv_chunk(j)
                    emit_attention(j)
            elif prio_mode:
                # emit qkv first (program order = dataflow order), but give
                # attention tj a priority window starting right after qkv
                # chunk tj, so the scheduler fills attention stalls with
                # later qkv chunks
                cp = []
                for j in range(NTJ):
                    emit_qkv_chunk(j)
                    cp.append(tc.cur_priority)
                if pre3:
                    # tj3's S/exp/mask precompute as mid-kernel filler
                    # (window right after qkv chunk 3); its PV/norm/proj
                    # run last as a dense pure-PE tail
                    for j in range(NTJ - 1):
                        off = tc.cur_priority - cp[j]
                        with tc.high_priority(offset=off):
                            emit_attention(j)
                    off = tc.cur_priority - cp[NTJ - 1]
                    with tc.high_priority(offset=off):
                        emit_attention(NTJ - 1, phase="scores")
                    emit_attention(NTJ - 1, phase="pv")
                else:
                    for j in range(NTJ):
                        off = tc.cur_priority - cp[j]
                        with tc.high_priority(offset=off):
                            emit_attention(j)
            else:
                for j in range(NTJ):
                    emit_qkv_chunk(j)
                for j in range(NTJ):
                    emit_attention(j)
    return nc


_NC_CACHE: dict = {}


def _use_fp8(mask_mode: str, has_qkv_bias: bool) -> bool:
    if os.environ.get("K_FP8", "1") != "1":
        return False
    return mask_mode == "causal" and not has_qkv_bias


def _get_nc(mask_mode: str, has_qkv_bias: bool):
    if _use_fp8(mask_mode, has_qkv_bias):
        key = "fp8"
        if key not in _NC_CACHE:
            _NC_CACHE[key] = _build_fp8()
        return _NC_CACHE[key]
    key = (mask_mode, has_qkv_bias)
    if key not in _NC_CACHE:
        _NC_CACHE[key] = _build(mask_mode, has_qkv_bias)
    return _NC_CACHE[key]


def classify_inputs(mask, qkv_b):
    m2 = np.asarray(mask).reshape(T, T)
    if np.array_equal(m2 != 0, np.tril(np.ones((T, T), dtype=bool))):
        mask_mode = "causal"
    elif np.all(m2 != 0):
        mask_mode = "none"
    else:
        mask_mode = "generic"
    has_qkv_bias = bool(np.any(np.asarray(qkv_b) != 0.0))
    return mask_mode, has_qkv_bias


def prepare_in_maps(x, mask, qkv_w, qkv_b, proj_w, proj_b):
    x = np.asarray(x, dtype=np.float32)
    qkv_w = np.asarray(qkv_w, dtype=np.float32)
    qkv_b = np.asarray(qkv_b, dtype=np.float32)
    proj_w = np.asarray(proj_w, dtype=np.float32)
    mask_mode, has_qkv_bias = classify_inputs(mask, qkv_b)
    m2 = np.asarray(mask).reshape(T, T)

    tri_np = np.triu(np.ones((128, 128))).astype(NP_BF16)
    in_maps = []
    for b in range(B):
        xT_b = np.ascontiguousarray(x[b].T).astype(NP_BF16)
        for g in range(G):
            qs = qkv_w[:, g * VC : (g + 1) * VC]
            ks = qkv_w[:, D_MODEL + g * VC : D_MODEL + (g + 1) * VC]
            vs = qkv_w[:, 2 * D_MODEL + g * VC : 2 * D_MODEL + (g + 1) * VC]
            im = {
                "xT": xT_b,
                "wqkv": np.ascontiguousarray(
                    np.concatenate([qs, ks, vs], axis=1)
                ).astype(NP_BF16),
                "wproj": np.ascontiguousarray(
                    proj_w[g * VC : (g + 1) * VC, :]
                ).astype(NP_BF16),
            }
            if mask_mode == "causal":
                im["tri"] = tri_np
            if mask_mode == "generic":
                im["maskT"] = np.ascontiguousarray(
                    (m2 != 0).T.astype(NP_BF16)
                )
            if has_qkv_bias:
                im["bqk"] = np.ascontiguousarray(
                    np.concatenate(
                        [qkv_b[g * VC : (g + 1) * VC],
                         qkv_b[D_MODEL + g * VC : D_MODEL + (g + 1) * VC]]
                    )
                ).astype(np.float32)
                im["bv"] = np.ascontiguousarray(
                    qkv_b[2 * D_MODEL + g * VC : 2 * D_MODEL + (g + 1) * VC]
                ).astype(np.float32)
            in_maps.append(im)
    return in_maps


def prepare_in_maps_fp8(x, qkv_w, proj_w):
    x = np.asarray(x, dtype=np.float32)
    qkv_w = np.asarray(qkv_w, dtype=np.float32) * S_W
    proj_w = np.asarray(proj_w, dtype=np.float32) / S_W
    tri_np = np.triu(np.ones((128, 128))).astype(NP_BF16)
    zeros_np = np.zeros((64, HPC * T), dtype=NP_F8)
    in_maps = []
    for b in range(B):
        xT = np.ascontiguousarray(x[b].T)
        xh = xT.astype(NP_F8)
        xl = (xT - xh.astype(np.float32)).astype(NP_F8)
        for g in range(G):
            qs = qkv_w[:, g * VC : (g + 1) * VC]
            ks = qkv_w[:, D_MODEL + g * VC : D_MODEL + (g + 1) * VC]
            vs = qkv_w[:, 2 * D_MODEL + g * VC : 2 * D_MODEL + (g + 1) * VC]
            w = np.ascontiguousarray(np.concatenate([qs, ks, vs], axis=1))
            wh = w.astype(NP_F8)
            wl = (w - wh.astype(np.float32)).astype(NP_F8)
            in_maps.append(
                {
                    "xh": xh,
                    "xl": xl,
                    "wh": wh,
                    "wl": wl,
                    "wproj": np.ascontiguousarray(
                        proj_w[g * VC : (g + 1) * VC, :]
                    ).astype(NP_BF16),
                    "tri": tri_np,
                    "zeros": zeros_np,
                }
            )
    return in_maps


def kernel(x, mask, qkv_w, qkv_b, proj_w, proj_b):
    proj_b = np.asarray(proj_b, dtype=np.float32)
    mask_mode, has_qkv_bias = classify_inputs(mask, qkv_b)
    nc = _get_nc(mask_mode, has_qkv_bias)
    if _use_fp8(mask_mode, has_qkv_bias):
        in_maps = prepare_in_maps_fp8(x, qkv_w, proj_w)
    else:
        in_maps = prepare_in_maps(x, mask, qkv_w, qkv_b, proj_w, proj_b)

    trace = bool(os.environ.get("KERNEL_TRACE"))
    res = run_bass_kernel_spmd(
        nc, in_maps, core_ids=list(range(B * G)), trace=trace
    )
    globals()["LAST_RESULT"] = res
    outs = [np.asarray(r["out"], dtype=np.float32) for r in res.results]

    final = np.empty((B, T, D_MODEL), dtype=np.float32)
    for b in range(B):
        acc = outs[b * G].copy()
        for g in range(1, G):
            acc += outs[b * G + g]
        final[b] = acc + proj_b[None, :]
    return final

